# revision 8
# baseline (speedup 1.0000x reference)
"""GCN (3-layer + linear head) Trainium2 Bass kernel, sharded over 8 NeuronCores.

v2 strategy (vertex partitioning per the sharding hint, all-bf16 datapath):
 - Nodes sharded contiguously: core c owns nodes [c*12500, (c+1)*12500),
   padded to 12544 = 98 blocks of 128 rows.
 - Per layer: transform y = dinv^k * (h @ W) runs directly off the
   transposed hidden state hT kept in SBUF (no PE transposes anywhere);
   y shards are AllGathered part-wise (4 partial collectives per layer,
   bf16) so the halo exchange overlaps the transform and aggregation.
 - Aggregation gathers per-edge source rows (dma_gather, 256B bf16 rows)
   and realizes the segment-sum as one-hot matmuls accumulating into
   per-target-block PSUM banks. One-hot builds are batched per segment
   with a single broadcast-AP is_equal on the vector engine.
 - Self-loop messages skip the gather entirely: they are applied as a
   diag(dinv) matmul against the local y block in SBUF.
 - The hidden state is stored unscaled (relu(dinv*(x))=dinv*relu(x));
   dinv factors fold into the next transform (dinv^2) and the final
   projection (dinv), both per-partition activation scales.
 - Host does integer index prep only: degree counts, edge sorting by
   (core, group, part, block, source), structural chunk padding shared
   across cores so all 8 run one SPMD instruction stream.
"""
import os
import sys

sys.path.insert(0, "/opt/trn_rl_repo")

import numpy as np
import ml_dtypes

_NLAYERS = int(os.environ.get("GCN_NLAYERS", "3"))
_SKIP_AGG = bool(int(os.environ.get("GCN_SKIP_AGG", "0")))
_SKIP_FINAL = bool(int(os.environ.get("GCN_SKIP_FINAL", "0")))
_NO_MM = bool(int(os.environ.get("GCN_NO_MM", "0")))
_NO_SBUILD = bool(int(os.environ.get("GCN_NO_SBUILD", "0")))
_ONECORE = bool(int(os.environ.get("GCN_ONECORE", "0")))
_REPEAT = int(os.environ.get("GCN_REPEAT", "1"))
_NAG = int(os.environ.get("GCN_NAG", "4"))       # parts per AllGather split
_NO_AG = bool(int(os.environ.get("GCN_NO_AG", "0")))
_ILV = bool(int(os.environ.get("GCN_INTERLEAVE", "0")))
_FUSEPROJ = bool(int(os.environ.get("GCN_FUSEPROJ", "1")))
_NQUEUE = int(os.environ.get("GCN_NQUEUE", "4"))
_MPOOL = int(os.environ.get("GCN_MPOOL", "9"))
_PPA = int(os.environ.get("GCN_PPA", "4"))
_SPOOL = int(os.environ.get("GCN_SPOOL", "3"))
_YPOOL = int(os.environ.get("GCN_YPOOL", "4"))

import concourse.bacc as bacc
import concourse.mybir as mybir
import concourse.tile as tile
from concourse import bass_utils
from concourse.bass import AP
from concourse.library_config import mlp

# Problem constants (hardcoded per harness contract).
N_NODES = 100000
N_EDGES = 1600000
D = 128
D_LAB = 10
NCORES = 8
SHARD = 12500
SHARD_P = 12544           # 98 blocks of 128 rows
B = SHARD_P // 128        # 98
G = 4                     # target blocks per aggregation group (psum banks)
NG = -(-B // G)           # 25 groups
PBLK = [25, 25, 25, 23]   # blocks per source part (AllGather split)
PSTART = [0, 25, 50, 75]  # first block of each part
PROWS = [p * 128 for p in PBLK]
NQ = 4                    # source parts (int16-indexable: 8*3200 = 25600)

F32 = mybir.dt.float32
BF16 = mybir.dt.bfloat16
I16 = mybir.dt.int16
AF = mybir.ActivationFunctionType
ALU = mybir.AluOpType

PAD_TGT = 200.0           # bf16-exact, outside [0,128): padded slots match nothing


QOFF = [0, 3200, 6400, 9600]  # row offset of each transform part in the stage


def _preprocess(edge_index):
    """Host-side integer/index prep. Returns per-core arrays + shared structure.

    v3: one AllGather per layer — the gathered buffer is core-major
    [core, stage_row]; gather source groups are core PAIRS (idx16 =
    (c_s % 2) * SHARD_P + stage_row <= 25087, int16-safe) with the
    dma_gather base AP offset selecting the pair."""
    src = np.asarray(edge_index[0], dtype=np.int64)
    tgt = np.asarray(edge_index[1], dtype=np.int64)

    # degree includes the self-loop (handled on-device via diag matmul)
    deg = (np.bincount(tgt, minlength=N_NODES) + 1).astype(np.float64)

    c_t = tgt // SHARD
    tl = tgt % SHARD
    blk = tl // 128
    grp = blk // G
    c_s = src // SHARD
    r = src % SHARD
    rq = np.minimum(r // 3200, 3)
    pstart = np.asarray(PSTART, np.int64)[rq]
    pblk = np.asarray(PBLK, np.int64)[rq]
    qoff = np.asarray(QOFF, np.int64)[rq]
    stage_row = qoff + (r % 128) * pblk + (r // 128 - pstart)
    q = c_s // 2  # gather source group = core pair
    qrel = (c_s % 2) * SHARD_P + stage_row

    order = np.lexsort((qrel, blk, q, grp, c_t))
    c_t, tl, blk, q, qrel = c_t[order], tl[order], blk[order], q[order], qrel[order]

    # segment = (core, g, q, blk); structural chunk count = max over cores
    seg_key = ((c_t * NG + (blk // G)) * NQ + q) * B + blk
    nseg = NCORES * NG * NQ * B
    counts = np.bincount(seg_key, minlength=nseg).reshape(NCORES, NG, NQ, B)
    C = (-(-counts // 128)).max(axis=0)  # [NG, NQ, B]

    tot_chunks = int(C.sum())
    TOT = tot_chunks * 128
    idx_all = np.zeros((NCORES, TOT), dtype=np.int16)
    tgt_all = np.full((NCORES, TOT), PAD_TGT, dtype=np.float32)

    seg_starts = np.zeros(nseg + 1, dtype=np.int64)
    np.cumsum(np.bincount(seg_key, minlength=nseg), out=seg_starts[1:])

    segs = []       # (g, q, b, nch, ci0) in emission order
    nch_gq = np.zeros((NG, NQ), dtype=np.int64)
    off = 0
    for g in range(NG):
        for qq in range(NQ):
            for b in range(g * G, min((g + 1) * G, B)):
                nch = int(C[g, qq, b])
                if nch == 0:
                    continue
                segs.append((g, qq, b, nch, off // 128))
                nch_gq[g, qq] += nch
                for c in range(NCORES):
                    k = ((c * NG + g) * NQ + qq) * B + b
                    s0, s1 = seg_starts[k], seg_starts[k + 1]
                    n = s1 - s0
                    idx_all[c, off:off + n] = qrel[s0:s1]
                    tgt_all[c, off:off + n] = (tl[s0:s1] - b * 128)
                off += nch * 128
    assert off == TOT

    idx_wrapped = np.stack([
        np.tile(a.reshape(-1, 16).T, (8, 1)) for a in idx_all])
    tgt_tiles = np.ascontiguousarray(
        tgt_all.reshape(NCORES, tot_chunks, 128).transpose(0, 2, 1)
    ).astype(ml_dtypes.bfloat16)

    deg_p = np.ones((NCORES, SHARD_P), dtype=np.float64)
    deg_p[:, :SHARD] = deg.reshape(NCORES, SHARD)
    dinv = 1.0 / np.sqrt(deg_p)
    dinv_col = np.ascontiguousarray(
        dinv.reshape(NCORES, B, 128).transpose(0, 2, 1)).astype(np.float32)
    dinv2_col = np.ascontiguousarray(
        (1.0 / deg_p).reshape(NCORES, B, 128).transpose(0, 2, 1)).astype(np.float32)
    sqdeg_row = np.sqrt(deg_p).reshape(NCORES, 1, SHARD_P).astype(np.float32)

    return dict(idx=idx_wrapped, tgt=tgt_tiles, dinv_col=dinv_col,
                dinv2_col=dinv2_col, sqdeg_row=sqdeg_row,
                segs=segs, nch_gq=nch_gq, tot_chunks=tot_chunks, TOT=TOT)


def _build(pre):
    """Build the Bass/Tile program (one SPMD NEFF for all 8 cores)."""
    TOT = pre["TOT"]
    tot_chunks = pre["tot_chunks"]
    nch_gq = pre["nch_gq"]
    segs = pre["segs"]

    nc = bacc.Bacc("TRN2", target_bir_lowering=False, debug=False,
                   num_devices=1 if _ONECORE else NCORES,
                   num_swdge_queues=_NQUEUE)

    featT_d = nc.dram_tensor("featT", [D, SHARD_P], BF16, kind="ExternalInput")
    idx_d = nc.dram_tensor("idx", [128, TOT // 16], I16, kind="ExternalInput")
    tgt_d = nc.dram_tensor("tgt", [128, tot_chunks], BF16, kind="ExternalInput")
    dinv_d = nc.dram_tensor("dinv_col", [128, B], F32, kind="ExternalInput")
    dinv2_d = nc.dram_tensor("dinv2_col", [128, B], F32, kind="ExternalInput")
    sqdeg_d = nc.dram_tensor("sqdeg_row", [1, SHARD_P], F32, kind="ExternalInput")
    w_d = nc.dram_tensor("w_all", [128, 3 * D], BF16, kind="ExternalInput")
    b_d = nc.dram_tensor("b_all", [1, 3 * D], F32, kind="ExternalInput")
    wp_d = nc.dram_tensor("wp_all", [128, 3 * D_LAB], BF16, kind="ExternalInput")
    bp_d = nc.dram_tensor("bp", [1, D_LAB], F32, kind="ExternalInput")
    iota_d = nc.dram_tensor("iota", [128, 128], BF16, kind="ExternalInput")
    ident_d = nc.dram_tensor("ident", [128, 128], BF16, kind="ExternalInput")

    out_d = nc.dram_tensor("out", [SHARD_P, D_LAB], F32, kind="ExternalOutput")

    with tile.TileContext(nc) as tc:
        with (
            tc.tile_pool(name="const", bufs=1) as cpool,
            tc.tile_pool(name="work", bufs=3) as wpool,
            tc.tile_pool(name="hstate", bufs=(2 if _FUSEPROJ else 3) * NQ) as hpool,
            tc.tile_pool(name="ystate", bufs=_YPOOL) as ypool,
            tc.tile_pool(name="projacc", bufs=2) as apool,
            tc.tile_pool(name="sbuild", bufs=_SPOOL) as spool,
            tc.tile_pool(name="mtiles", bufs=_MPOOL) as mpool,
            tc.tile_pool(name="itiles", bufs=3) as ipool,
            tc.tile_pool(name="psum_a", bufs=_PPA, space="PSUM") as ppa,
            tc.tile_pool(name="psum_y", bufs=2, space="PSUM") as ppy,
            tc.tile_pool(name="dram", bufs=1, space="DRAM") as dpool,
        ):
            nc.gpsimd.load_library(mlp)

            # ---- constants ----
            tgt_s = cpool.tile([128, tot_chunks], BF16)
            iota_s = cpool.tile([128, 128], BF16)
            ident_s = cpool.tile([128, 128], BF16)
            w_s = cpool.tile([128, 3 * D], BF16)
            b_s = cpool.tile([1, 3 * D], F32)
            wp_s = cpool.tile([128, 3 * D_LAB], BF16)
            bp_s = cpool.tile([1, D_LAB], F32)
            dinv_s = cpool.tile([128, B], F32)
            dinv2_s = cpool.tile([128, B], F32)
            sqdeg_s = cpool.tile([1, SHARD_P], F32)

            nc.sync.dma_start(tgt_s[:], tgt_d[:])
            nc.sync.dma_start(iota_s[:], iota_d[:])
            nc.sync.dma_start(ident_s[:], ident_d[:])
            nc.sync.dma_start(w_s[:], w_d[:])
            nc.sync.dma_start(b_s[:], b_d[:])
            nc.sync.dma_start(wp_s[:], wp_d[:])
            nc.sync.dma_start(bp_s[:], bp_d[:])
            nc.sync.dma_start(dinv_s[:], dinv_d[:])
            nc.sync.dma_start(dinv2_s[:], dinv2_d[:])
            nc.sync.dma_start(sqdeg_s[:], sqdeg_d[:])

            # ---- internal DRAM: one AllGather landing buffer per (rep,layer)
            y_fulls = {}
            y_stage = {}
            for rep in range(_REPEAT):
                for layer in range(_NLAYERS):
                    y_fulls[(rep, layer)] = dpool.tile(
                        [NCORES * SHARD_P, D], BF16,
                        addr_space=os.environ.get("GCN_YPART_SPACE",
                                                  "Shared"),
                        name=f"y_full_{rep}_{layer}")
                    y_stage[(rep, layer)] = dpool.tile(
                        [SHARD_P, D], BF16,
                        name=f"y_stage_{rep}_{layer}")

            hT = {}          # (layer, qq) -> SBUF tile [128 f, PROWS[q] cols]
            qrr = 0          # gather queue round-robin

            def seg_onehot(st_t, ci0, nch):
                """st[:, k*128+j] = (iota[:, j] == tgt[:, ci0+k]), batched."""
                o = AP(st_t[:, :].tensor, st_t[:, :].offset,
                       [st_t[:, :].ap[0], [128, nch], [1, 128]])
                i1 = AP(iota_s[:, :].tensor, iota_s[:, :].offset,
                        [iota_s[:, :].ap[0], [0, nch], [1, 128]])
                t2 = tgt_s[:, ci0:ci0 + nch]
                i2 = AP(t2.tensor, t2.offset, [t2.ap[0], [1, nch], [0, 128]])
                nc.vector.tensor_tensor(o, i1, i2, ALU.is_equal)

            def emit_transform_part(rep, layer, qq):
                """y = dinv^k * (hT_prev^T @ W) for part qq -> stage rows."""
                wl = w_s[:, layer * D:(layer + 1) * D]
                y_q = ypool.tile([128, PROWS[qq]], BF16, tag="ysb",
                                 name=f"y_{rep}_{layer}_{qq}")
                for j in range(PBLK[qq]):
                    b = PSTART[qq] + j
                    if layer == 0:
                        fb = wpool.tile([128, 128], BF16, tag="fb")
                        nc.sync.dma_start(
                            fb[:], featT_d[:, b * 128:(b + 1) * 128])
                        stat = fb[:]
                    else:
                        stat = hT[(rep, layer - 1, qq)][:, j * 128:(j + 1) * 128]
                    yp = ppy.tile([128, 128], F32, tag="ty")
                    nc.tensor.matmul(yp[:], stat, wl, start=True, stop=True)
                    scale = (dinv_s if layer == 0 else dinv2_s)[:, b:b + 1]
                    nc.scalar.activation(y_q[:, j * 128:(j + 1) * 128],
                                         yp[:], AF.Copy,
                                         bias=0.0, scale=scale)
                dst = (y_fulls[(rep, layer)] if _ONECORE
                       else y_stage[(rep, layer)])
                nc.sync.dma_start(
                    dst[QOFF[qq]:QOFF[qq] + PROWS[qq], :], y_q[:])
                hT[("y", layer, qq)] = y_q

            def emit_allgather(rep, layer):
                if _ONECORE or _NO_AG:
                    return
                nc.gpsimd.collective_compute(
                    "AllGather", ALU.bypass,
                    replica_groups=[list(range(NCORES))],
                    ins=[y_stage[(rep, layer)].opt()],
                    outs=[y_fulls[(rep, layer)].opt()],
                )

            def emit_agg_group(rep, layer, g, cur):
                nonlocal qrr
                bl = b_s[:, layer * D:(layer + 1) * D]
                blocks = list(range(g * G, min((g + 1) * G, B)))
                psums = {}
                for b in blocks:
                    ps = ppa.tile([128, 128], F32, tag="agg",
                                  name=f"ps_{rep}_{layer}_{b}")
                    psums[b] = ps
                    # open the accumulation group with the bias rank-1
                    r0, r1 = b * 128, (b + 1) * 128
                    nc.tensor.matmul(ps[:], bl, sqdeg_s[:, r0:r1],
                                     start=True, stop=False)
                nch_g = int(nch_gq[g].sum())
                g16_0 = segs[cur[0]][4] * 8
                idx_g = ipool.tile([128, nch_g * 8], I16, tag="idxs",
                                   name=f"ix_{rep}_{layer}_{g}")
                nc.sync.dma_start(idx_g[:], idx_d[:, g16_0:g16_0 + nch_g * 8])
                y_full = y_fulls[(rep, layer)]
                for qq in range(NQ):
                    nch = int(nch_gq[g, qq])
                    if nch == 0:
                        continue
                    ci0 = segs[cur[0]][4]
                    n_idx = nch * 128
                    n16 = n_idx // 16
                    off16 = ci0 * 8 - g16_0
                    mt = mpool.tile([128, nch, 128], BF16, tag="m",
                                    name=f"m_{rep}_{layer}_{g}_{qq}")
                    src = y_full[qq * 2 * SHARD_P:(qq + 1) * 2 * SHARD_P, :]
                    nc.gpsimd.dma_gather(
                        mt[:], src, idx_g[:, off16:off16 + n16],
                        n_idx, n_idx, D, single_packet=False,
                        queue_num=qrr % _NQUEUE)
                    qrr += 1
                    if not _NO_SBUILD:
                        # one batched is_equal for all chunks of this (g,q)
                        st_t = spool.tile([128, nch * 128], BF16, tag="s",
                                          name=f"s_{rep}_{layer}_{g}_{qq}")
                        seg_onehot(st_t, ci0, nch)
                    k = 0
                    while k < nch:
                        g2, q2, b2, nck, ci = segs[cur[0]]
                        assert g2 == g and q2 == qq and ci == ci0 + k
                        if not _NO_MM:
                            for u in range(nck):
                                nc.tensor.matmul(
                                    psums[b2][:], mt[:, k + u, :],
                                    st_t[:, (k + u) * 128:(k + u + 1) * 128],
                                    start=False, stop=False)
                        k += nck
                        cur[0] += 1
                # close each block: self-loop identity matmul
                # (h = dinv*hTu, so the self message dinv^2*(hW) is just y)
                for b in blocks:
                    pq = min(b // 25, 3)
                    pj = b - PSTART[pq]
                    y_q = hT[("y", layer, pq)]
                    nc.tensor.matmul(
                        psums[b][:], y_q[:, pj * 128:(pj + 1) * 128],
                        ident_s[:], start=False, stop=True)
                for b in blocks:
                    pq = min(b // 25, 3)
                    pj = b - PSTART[pq]
                    if (rep, layer, pq) not in hT:
                        hT[(rep, layer, pq)] = hpool.tile(
                            [128, PROWS[pq]], BF16, tag="h",
                            name=f"h_{rep}_{layer}_{pq}")
                    func = AF.Relu if layer < _NLAYERS - 1 else AF.Copy
                    h_blk = hT[(rep, layer, pq)][:, pj * 128:(pj + 1) * 128]
                    nc.scalar.activation(h_blk, psums[b][:], func)
                    if _FUSEPROJ and do_proj:
                        # fold this layer's projection term into the
                        # accumulator so hTu can be dropped after use
                        r0, r1 = b * 128, (b + 1) * 128
                        pf = ppy.tile([128, D_LAB], F32, tag="pf",
                                      name=f"pf_{rep}_{layer}_{b}")
                        nc.tensor.matmul(
                            pf[:], h_blk,
                            wp_s[:, layer * D_LAB:(layer + 1) * D_LAB],
                            start=True, stop=(layer != 0))
                        if layer == 0:
                            nc.tensor.matmul(pf[:], sqdeg_s[:, r0:r1],
                                             bp_s[:], start=False, stop=True)
                        acc_sl = proj_acc[:, b * D_LAB:(b + 1) * D_LAB]
                        if layer == 0:
                            nc.vector.tensor_copy(acc_sl, pf[:])
                        else:
                            nc.vector.tensor_add(acc_sl, acc_sl, pf[:])

            def emit_projection_part(rep, qq):
                """out[t, l] = dinv[t]*(sum_i hTu_i^T Wp_i + sqrt(deg) bp)."""
                for j in range(PBLK[qq]):
                    b = PSTART[qq] + j
                    r0, r1 = b * 128, (b + 1) * 128
                    pf = ppy.tile([128, D_LAB], F32, tag="pf",
                                  name=f"pf_{rep}_{b}")
                    nc.tensor.matmul(pf[:], sqdeg_s[:, r0:r1], bp_s[:],
                                     start=True, stop=False)
                    for i in range(3):
                        nc.tensor.matmul(
                            pf[:], hT[(i, qq)][:, j * 128:(j + 1) * 128],
                            wp_s[:, i * D_LAB:(i + 1) * D_LAB],
                            start=False, stop=(i == 2))
                    fo = wpool.tile([128, D_LAB], F32, tag="fo")
                    nc.scalar.activation(fo[:], pf[:], AF.Copy,
                                         bias=0.0, scale=dinv_s[:, b:b + 1])
                    nc.sync.dma_start(out_d[r0:r1, :], fo[:])

            # part qq's hTu is complete after this aggregation group closes
            PART_READY = {(PSTART[p] + PBLK[p] - 1) // G: p for p in range(NQ)}
            do_proj = not (_SKIP_FINAL or _NLAYERS < 3)

            for rep in range(_REPEAT):
                if _FUSEPROJ and do_proj:
                    proj_acc = apool.tile([128, B * D_LAB], F32, tag="acc",
                                          name=f"acc_{rep}")
                for qq in range(NQ):
                    emit_transform_part(rep, 0, qq)
                emit_allgather(rep, 0)
                for layer in range(_NLAYERS):
                    if not _ILV and layer > 0:
                        for qq in range(NQ):
                            emit_transform_part(rep, layer, qq)
                        emit_allgather(rep, layer)
                    if _SKIP_AGG:
                        for qq in range(NQ):
                            h_q = hpool.tile([128, PROWS[qq]], BF16, tag="h",
                                             name=f"h_{rep}_{layer}_{qq}")
                            nc.vector.tensor_copy(h_q[:],
                                                  hT[("y", layer, qq)][:])
                            hT[(rep, layer, qq)] = h_q
                            if layer + 1 < _NLAYERS:
                                emit_transform_part(rep, layer + 1, qq)
                                if qq == NQ - 1:
                                    emit_allgather(rep, layer + 1)
                        continue
                    cur = [0]
                    for g in range(NG):
                        emit_agg_group(rep, layer, g, cur)
                        p = PART_READY.get(g)
                        if _ILV and p is not None:
                            # interleave next-layer transform (or projection)
                            # so its stage writes hide behind this aggregation
                            if layer + 1 < _NLAYERS:
                                emit_transform_part(rep, layer + 1, p)
                                if p == NQ - 1:
                                    emit_allgather(rep, layer + 1)
                            elif (layer == _NLAYERS - 1 and do_proj
                                  and not _FUSEPROJ):
                                emit_projection_part(rep, p)
                    assert cur[0] == len(segs)
                if not _ILV and do_proj and not _FUSEPROJ:
                    for qq in range(NQ):
                        emit_projection_part(rep, qq)
                if _FUSEPROJ and do_proj:
                    for b in range(B):
                        r0, r1 = b * 128, (b + 1) * 128
                        fo = wpool.tile([128, D_LAB], F32, tag="fo")
                        nc.scalar.activation(
                            fo[:], proj_acc[:, b * D_LAB:(b + 1) * D_LAB],
                            AF.Copy, bias=0.0, scale=dinv_s[:, b:b + 1])
                        nc.sync.dma_start(out_d[r0:r1, :], fo[:])

    nc.compile()
    return nc


_CACHE = {}


def _get_program(edge_index):
    key = hash(np.asarray(edge_index).tobytes())
    if key not in _CACHE:
        pre = _preprocess(edge_index)
        nc = _build(pre)
        _CACHE.clear()
        _CACHE[key] = (pre, nc)
    return _CACHE[key]


def prepare(feat, edge_index, W1, b1, W2, b2, W3, b3, Wp, bp):
    """Build (nc, in_maps) for the SPMD run."""
    feat = np.asarray(feat, np.float32)
    edge_index = np.asarray(edge_index, np.int32)
    W1, b1, W2, b2, W3, b3, Wp, bp = (np.asarray(a, np.float32)
                                      for a in (W1, b1, W2, b2, W3, b3, Wp, bp))
    pre, nc = _get_program(edge_index)

    w_all = np.concatenate([W1, W2, W3], axis=1).astype(ml_dtypes.bfloat16)
    b_all = np.concatenate([b1, b2, b3]).reshape(1, 3 * D).astype(np.float32)
    wp_all = np.concatenate([Wp[:D], Wp[D:2 * D], Wp[2 * D:]],
                            axis=1).astype(ml_dtypes.bfloat16)
    iota = np.broadcast_to(np.arange(128, dtype=np.float32),
                           (128, 128)).astype(ml_dtypes.bfloat16)
    ident = np.eye(128, dtype=np.float32).astype(ml_dtypes.bfloat16)

    feat_p = np.zeros((NCORES, SHARD_P, D), np.float32)
    feat_p[:, :SHARD] = feat.reshape(NCORES, SHARD, D)
    featT = np.ascontiguousarray(
        feat_p.transpose(0, 2, 1)).astype(ml_dtypes.bfloat16)

    in_maps = []
    for c in range(NCORES):
        in_maps.append({
            "featT": featT[c],
            "idx": pre["idx"][c],
            "tgt": pre["tgt"][c],
            "dinv_col": pre["dinv_col"][c],
            "dinv2_col": pre["dinv2_col"][c],
            "sqdeg_row": pre["sqdeg_row"][c],
            "w_all": w_all, "b_all": b_all,
            "wp_all": wp_all, "bp": bp.reshape(1, D_LAB).astype(np.float32),
            "iota": iota, "ident": ident,
        })
    return nc, in_maps


def kernel(**inputs):
    nc, in_maps = prepare(**inputs)
    trace = bool(int(os.environ.get("GCN_TRACE", "0")))
    res = bass_utils.run_bass_kernel_spmd(nc, in_maps,
                                          core_ids=list(range(NCORES)),
                                          trace=trace)
    global LAST_RESULTS
    LAST_RESULTS = res
    out = np.empty((N_NODES, D_LAB), np.float32)
    for c in range(NCORES):
        out[c * SHARD:(c + 1) * SHARD] = res.results[c]["out"][:SHARD]
    return out


LAST_RESULTS = None



# revision 9
# speedup vs baseline: 1.0285x; 1.0285x over previous
"""GCN (3-layer + linear head) Trainium2 Bass kernel, sharded over 8 NeuronCores.

v2 strategy (vertex partitioning per the sharding hint, all-bf16 datapath):
 - Nodes sharded contiguously: core c owns nodes [c*12500, (c+1)*12500),
   padded to 12544 = 98 blocks of 128 rows.
 - Per layer: transform y = dinv^k * (h @ W) runs directly off the
   transposed hidden state hT kept in SBUF (no PE transposes anywhere);
   y shards are AllGathered part-wise (4 partial collectives per layer,
   bf16) so the halo exchange overlaps the transform and aggregation.
 - Aggregation gathers per-edge source rows (dma_gather, 256B bf16 rows)
   and realizes the segment-sum as one-hot matmuls accumulating into
   per-target-block PSUM banks. One-hot builds are batched per segment
   with a single broadcast-AP is_equal on the vector engine.
 - Self-loop messages skip the gather entirely: they are applied as a
   diag(dinv) matmul against the local y block in SBUF.
 - The hidden state is stored unscaled (relu(dinv*(x))=dinv*relu(x));
   dinv factors fold into the next transform (dinv^2) and the final
   projection (dinv), both per-partition activation scales.
 - Host does integer index prep only: degree counts, edge sorting by
   (core, group, part, block, source), structural chunk padding shared
   across cores so all 8 run one SPMD instruction stream.
"""
import os
import sys

sys.path.insert(0, "/opt/trn_rl_repo")

import numpy as np
import ml_dtypes

_NLAYERS = int(os.environ.get("GCN_NLAYERS", "3"))
_SKIP_AGG = bool(int(os.environ.get("GCN_SKIP_AGG", "0")))
_SKIP_FINAL = bool(int(os.environ.get("GCN_SKIP_FINAL", "0")))
_NO_MM = bool(int(os.environ.get("GCN_NO_MM", "0")))
_NO_SBUILD = bool(int(os.environ.get("GCN_NO_SBUILD", "0")))
_ONECORE = bool(int(os.environ.get("GCN_ONECORE", "0")))
_REPEAT = int(os.environ.get("GCN_REPEAT", "1"))
_NAG = int(os.environ.get("GCN_NAG", "4"))       # parts per AllGather split
_NO_AG = bool(int(os.environ.get("GCN_NO_AG", "0")))
_ILV = bool(int(os.environ.get("GCN_INTERLEAVE", "0")))
_FUSEPROJ = bool(int(os.environ.get("GCN_FUSEPROJ", "1")))
_NQUEUE = int(os.environ.get("GCN_NQUEUE", "4"))
_MPOOL = int(os.environ.get("GCN_MPOOL", "9"))
_PPA = int(os.environ.get("GCN_PPA", "4"))
_SPOOL = int(os.environ.get("GCN_SPOOL", "3"))
_YPOOL = int(os.environ.get("GCN_YPOOL", "4"))

import concourse.bacc as bacc
import concourse.mybir as mybir
import concourse.tile as tile
from concourse import bass_utils
from concourse.bass import AP
from concourse.library_config import mlp

# Problem constants (hardcoded per harness contract).
N_NODES = 100000
N_EDGES = 1600000
D = 128
D_LAB = 10
NCORES = 8
SHARD = 12500
SHARD_P = 12544           # 98 blocks of 128 rows
B = SHARD_P // 128        # 98
G = 4                     # target blocks per aggregation group (psum banks)
NG = -(-B // G)           # 25 groups
PBLK = [25, 25, 25, 23]   # blocks per source part (AllGather split)
PSTART = [0, 25, 50, 75]  # first block of each part
PROWS = [p * 128 for p in PBLK]
NQ = 4                    # source parts (int16-indexable: 8*3200 = 25600)

F32 = mybir.dt.float32
BF16 = mybir.dt.bfloat16
I16 = mybir.dt.int16
AF = mybir.ActivationFunctionType
ALU = mybir.AluOpType

PAD_TGT = 200.0           # bf16-exact, outside [0,128): padded slots match nothing


QOFF = [0, 3200, 6400, 9600]  # row offset of each transform part in the stage


def _preprocess(edge_index):
    """Host-side integer/index prep. Returns per-core arrays + shared structure.

    v3: one AllGather per layer — the gathered buffer is core-major
    [core, stage_row]; gather source groups are core PAIRS (idx16 =
    (c_s % 2) * SHARD_P + stage_row <= 25087, int16-safe) with the
    dma_gather base AP offset selecting the pair."""
    src = np.asarray(edge_index[0], dtype=np.int64)
    tgt = np.asarray(edge_index[1], dtype=np.int64)

    # degree includes the self-loop (handled on-device via diag matmul)
    deg = (np.bincount(tgt, minlength=N_NODES) + 1).astype(np.float64)

    c_t = tgt // SHARD
    tl = tgt % SHARD
    blk = tl // 128
    grp = blk // G
    c_s = src // SHARD
    r = src % SHARD
    rq = np.minimum(r // 3200, 3)
    pstart = np.asarray(PSTART, np.int64)[rq]
    pblk = np.asarray(PBLK, np.int64)[rq]
    qoff = np.asarray(QOFF, np.int64)[rq]
    stage_row = qoff + (r % 128) * pblk + (r // 128 - pstart)
    q = c_s // 2  # gather source group = core pair
    qrel = (c_s % 2) * SHARD_P + stage_row

    order = np.lexsort((qrel, blk, q, grp, c_t))
    c_t, tl, blk, q, qrel = c_t[order], tl[order], blk[order], q[order], qrel[order]

    # segment = (core, g, q, blk); structural chunk count = max over cores
    seg_key = ((c_t * NG + (blk // G)) * NQ + q) * B + blk
    nseg = NCORES * NG * NQ * B
    counts = np.bincount(seg_key, minlength=nseg).reshape(NCORES, NG, NQ, B)
    C = (-(-counts // 128)).max(axis=0)  # [NG, NQ, B]

    tot_chunks = int(C.sum())
    TOT = tot_chunks * 128
    idx_all = np.zeros((NCORES, TOT), dtype=np.int16)
    tgt_all = np.full((NCORES, TOT), PAD_TGT, dtype=np.float32)

    seg_starts = np.zeros(nseg + 1, dtype=np.int64)
    np.cumsum(np.bincount(seg_key, minlength=nseg), out=seg_starts[1:])

    segs = []       # (g, q, b, nch, ci0) in emission order
    nch_gq = np.zeros((NG, NQ), dtype=np.int64)
    off = 0
    for g in range(NG):
        for qq in range(NQ):
            for b in range(g * G, min((g + 1) * G, B)):
                nch = int(C[g, qq, b])
                if nch == 0:
                    continue
                segs.append((g, qq, b, nch, off // 128))
                nch_gq[g, qq] += nch
                for c in range(NCORES):
                    k = ((c * NG + g) * NQ + qq) * B + b
                    s0, s1 = seg_starts[k], seg_starts[k + 1]
                    n = s1 - s0
                    idx_all[c, off:off + n] = qrel[s0:s1]
                    tgt_all[c, off:off + n] = (tl[s0:s1] - b * 128)
                off += nch * 128
    assert off == TOT

    idx_wrapped = np.stack([
        np.tile(a.reshape(-1, 16).T, (8, 1)) for a in idx_all])
    tgt_tiles = np.ascontiguousarray(
        tgt_all.reshape(NCORES, tot_chunks, 128).transpose(0, 2, 1)
    ).astype(ml_dtypes.bfloat16)

    deg_p = np.ones((NCORES, SHARD_P), dtype=np.float64)
    deg_p[:, :SHARD] = deg.reshape(NCORES, SHARD)
    dinv = 1.0 / np.sqrt(deg_p)
    dinv_col = np.ascontiguousarray(
        dinv.reshape(NCORES, B, 128).transpose(0, 2, 1)).astype(np.float32)
    dinv2_col = np.ascontiguousarray(
        (1.0 / deg_p).reshape(NCORES, B, 128).transpose(0, 2, 1)).astype(np.float32)
    sqdeg_row = np.sqrt(deg_p).reshape(NCORES, 1, SHARD_P).astype(np.float32)

    return dict(idx=idx_wrapped, tgt=tgt_tiles, dinv_col=dinv_col,
                dinv2_col=dinv2_col, sqdeg_row=sqdeg_row,
                segs=segs, nch_gq=nch_gq, tot_chunks=tot_chunks, TOT=TOT)


def _build(pre):
    """Build the Bass/Tile program (one SPMD NEFF for all 8 cores)."""
    TOT = pre["TOT"]
    tot_chunks = pre["tot_chunks"]
    nch_gq = pre["nch_gq"]
    segs = pre["segs"]

    nc = bacc.Bacc("TRN2", target_bir_lowering=False, debug=False,
                   num_devices=1 if _ONECORE else NCORES,
                   num_swdge_queues=_NQUEUE)

    featT_d = nc.dram_tensor("featT", [D, SHARD_P], BF16, kind="ExternalInput")
    idx_d = nc.dram_tensor("idx", [128, TOT // 16], I16, kind="ExternalInput")
    tgt_d = nc.dram_tensor("tgt", [128, tot_chunks], BF16, kind="ExternalInput")
    dinv_d = nc.dram_tensor("dinv_col", [128, B], F32, kind="ExternalInput")
    dinv2_d = nc.dram_tensor("dinv2_col", [128, B], F32, kind="ExternalInput")
    sqdeg_d = nc.dram_tensor("sqdeg_row", [1, SHARD_P], F32, kind="ExternalInput")
    w_d = nc.dram_tensor("w_all", [128, 3 * D], BF16, kind="ExternalInput")
    b_d = nc.dram_tensor("b_all", [1, 3 * D], F32, kind="ExternalInput")
    wp_d = nc.dram_tensor("wp_all", [128, 3 * D_LAB], BF16, kind="ExternalInput")
    bp_d = nc.dram_tensor("bp", [1, D_LAB], F32, kind="ExternalInput")
    iota_d = nc.dram_tensor("iota", [128, 128], BF16, kind="ExternalInput")
    ident_d = nc.dram_tensor("ident", [128, 128], BF16, kind="ExternalInput")

    out_d = nc.dram_tensor("out", [SHARD_P, D_LAB], F32, kind="ExternalOutput")

    with tile.TileContext(nc) as tc:
        with (
            tc.tile_pool(name="const", bufs=1) as cpool,
            tc.tile_pool(name="work", bufs=3) as wpool,
            tc.tile_pool(name="hstate", bufs=(2 if _FUSEPROJ else 3) * NQ) as hpool,
            tc.tile_pool(name="ystate", bufs=_YPOOL) as ypool,
            tc.tile_pool(name="projacc", bufs=2) as apool,
            tc.tile_pool(name="sbuild", bufs=_SPOOL) as spool,
            tc.tile_pool(name="mtiles", bufs=_MPOOL) as mpool,
            tc.tile_pool(name="itiles", bufs=3) as ipool,
            tc.tile_pool(name="psum_a", bufs=_PPA, space="PSUM") as ppa,
            tc.tile_pool(name="psum_y", bufs=2, space="PSUM") as ppy,
            tc.tile_pool(name="dram", bufs=1, space="DRAM") as dpool,
        ):
            nc.gpsimd.load_library(mlp)

            # ---- constants ----
            tgt_s = cpool.tile([128, tot_chunks], BF16)
            iota_s = cpool.tile([128, 128], BF16)
            ident_s = cpool.tile([128, 128], BF16)
            w_s = cpool.tile([128, 3 * D], BF16)
            b_s = cpool.tile([1, 3 * D], F32)
            wp_s = cpool.tile([128, 3 * D_LAB], BF16)
            bp_s = cpool.tile([1, D_LAB], F32)
            dinv_s = cpool.tile([128, B], F32)
            dinv2_s = cpool.tile([128, B], F32)
            sqdeg_s = cpool.tile([1, SHARD_P], F32)

            nc.sync.dma_start(tgt_s[:], tgt_d[:])
            nc.sync.dma_start(iota_s[:], iota_d[:])
            nc.sync.dma_start(ident_s[:], ident_d[:])
            nc.sync.dma_start(w_s[:], w_d[:])
            nc.sync.dma_start(b_s[:], b_d[:])
            nc.sync.dma_start(wp_s[:], wp_d[:])
            nc.sync.dma_start(bp_s[:], bp_d[:])
            nc.sync.dma_start(dinv_s[:], dinv_d[:])
            nc.sync.dma_start(dinv2_s[:], dinv2_d[:])
            nc.sync.dma_start(sqdeg_s[:], sqdeg_d[:])

            # ---- internal DRAM: one AllGather landing buffer per (rep,layer)
            y_fulls = {}
            y_stage = {}
            for rep in range(_REPEAT):
                for layer in range(_NLAYERS):
                    y_fulls[(rep, layer)] = dpool.tile(
                        [NCORES * SHARD_P, D], BF16,
                        addr_space=os.environ.get("GCN_YPART_SPACE",
                                                  "Shared"),
                        name=f"y_full_{rep}_{layer}")
                    y_stage[(rep, layer)] = dpool.tile(
                        [SHARD_P, D], BF16,
                        name=f"y_stage_{rep}_{layer}")

            hT = {}          # (layer, qq) -> SBUF tile [128 f, PROWS[q] cols]
            qrr = 0          # gather queue round-robin

            def seg_onehot(st_t, ci0, nch):
                """st[:, k*128+j] = (iota[:, j] == tgt[:, ci0+k]), batched."""
                o = AP(st_t[:, :].tensor, st_t[:, :].offset,
                       [st_t[:, :].ap[0], [128, nch], [1, 128]])
                i1 = AP(iota_s[:, :].tensor, iota_s[:, :].offset,
                        [iota_s[:, :].ap[0], [0, nch], [1, 128]])
                t2 = tgt_s[:, ci0:ci0 + nch]
                i2 = AP(t2.tensor, t2.offset, [t2.ap[0], [1, nch], [0, 128]])
                nc.vector.tensor_tensor(o, i1, i2, ALU.is_equal)

            def emit_transform_part(rep, layer, qq):
                """y = dinv^k * (hT_prev^T @ W) for part qq -> stage rows."""
                wl = w_s[:, layer * D:(layer + 1) * D]
                y_q = ypool.tile([128, PROWS[qq]], BF16, tag="ysb",
                                 name=f"y_{rep}_{layer}_{qq}")
                for j in range(PBLK[qq]):
                    b = PSTART[qq] + j
                    if layer == 0:
                        fb = wpool.tile([128, 128], BF16, tag="fb")
                        nc.sync.dma_start(
                            fb[:], featT_d[:, b * 128:(b + 1) * 128])
                        stat = fb[:]
                    else:
                        stat = hT[(rep, layer - 1, qq)][:, j * 128:(j + 1) * 128]
                    yp = ppy.tile([128, 128], F32, tag="ty")
                    nc.tensor.matmul(yp[:], stat, wl, start=True, stop=True)
                    scale = (dinv_s if layer == 0 else dinv2_s)[:, b:b + 1]
                    nc.scalar.activation(y_q[:, j * 128:(j + 1) * 128],
                                         yp[:], AF.Copy,
                                         bias=0.0, scale=scale)
                stage = y_stage[(rep, layer)]
                nc.sync.dma_start(
                    stage[QOFF[qq]:QOFF[qq] + PROWS[qq], :], y_q[:])
                hT[("y", layer, qq)] = y_q

            def emit_allgather(rep, layer):
                if _ONECORE:
                    nc.sync.dma_start(
                        y_fulls[(rep, layer)][0:SHARD_P, :],
                        y_stage[(rep, layer)][:])
                    return
                if _NO_AG:
                    return
                nc.gpsimd.collective_compute(
                    "AllGather", ALU.bypass,
                    replica_groups=[list(range(NCORES))],
                    ins=[y_stage[(rep, layer)].opt()],
                    outs=[y_fulls[(rep, layer)].opt()],
                )

            def emit_agg_group(rep, layer, g, cur):
                nonlocal qrr
                bl = b_s[:, layer * D:(layer + 1) * D]
                blocks = list(range(g * G, min((g + 1) * G, B)))
                psums = {}
                for b in blocks:
                    ps = ppa.tile([128, 128], F32, tag="agg",
                                  name=f"ps_{rep}_{layer}_{b}")
                    psums[b] = ps
                    # open the accumulation group with the bias rank-1
                    r0, r1 = b * 128, (b + 1) * 128
                    nc.tensor.matmul(ps[:], bl, sqdeg_s[:, r0:r1],
                                     start=True, stop=False)
                nch_g = int(nch_gq[g].sum())
                g16_0 = segs[cur[0]][4] * 8
                idx_g = ipool.tile([128, nch_g * 8], I16, tag="idxs",
                                   name=f"ix_{rep}_{layer}_{g}")
                nc.sync.dma_start(idx_g[:], idx_d[:, g16_0:g16_0 + nch_g * 8])
                y_full = y_fulls[(rep, layer)]
                for qq in range(NQ):
                    nch = int(nch_gq[g, qq])
                    if nch == 0:
                        continue
                    ci0 = segs[cur[0]][4]
                    n_idx = nch * 128
                    n16 = n_idx // 16
                    off16 = ci0 * 8 - g16_0
                    mt = mpool.tile([128, nch, 128], BF16, tag="m",
                                    name=f"m_{rep}_{layer}_{g}_{qq}")
                    src = y_full[qq * 2 * SHARD_P:(qq + 1) * 2 * SHARD_P, :]
                    nc.gpsimd.dma_gather(
                        mt[:], src, idx_g[:, off16:off16 + n16],
                        n_idx, n_idx, D, single_packet=False,
                        queue_num=qrr % _NQUEUE)
                    qrr += 1
                    if not _NO_SBUILD:
                        # one batched is_equal for all chunks of this (g,q)
                        st_t = spool.tile([128, nch * 128], BF16, tag="s",
                                          name=f"s_{rep}_{layer}_{g}_{qq}")
                        seg_onehot(st_t, ci0, nch)
                    k = 0
                    while k < nch:
                        g2, q2, b2, nck, ci = segs[cur[0]]
                        assert g2 == g and q2 == qq and ci == ci0 + k
                        if not _NO_MM:
                            for u in range(nck):
                                nc.tensor.matmul(
                                    psums[b2][:], mt[:, k + u, :],
                                    st_t[:, (k + u) * 128:(k + u + 1) * 128],
                                    start=False, stop=False)
                        k += nck
                        cur[0] += 1
                # close each block: self-loop identity matmul
                # (h = dinv*hTu, so the self message dinv^2*(hW) is just y)
                for b in blocks:
                    pq = min(b // 25, 3)
                    pj = b - PSTART[pq]
                    y_q = hT[("y", layer, pq)]
                    nc.tensor.matmul(
                        psums[b][:], y_q[:, pj * 128:(pj + 1) * 128],
                        ident_s[:], start=False, stop=True)
                for b in blocks:
                    pq = min(b // 25, 3)
                    pj = b - PSTART[pq]
                    if (rep, layer, pq) not in hT:
                        hT[(rep, layer, pq)] = hpool.tile(
                            [128, PROWS[pq]], BF16, tag="h",
                            name=f"h_{rep}_{layer}_{pq}")
                    func = AF.Relu if layer < _NLAYERS - 1 else AF.Copy
                    h_blk = hT[(rep, layer, pq)][:, pj * 128:(pj + 1) * 128]
                    nc.scalar.activation(h_blk, psums[b][:], func)
                    if _FUSEPROJ and do_proj:
                        # fold this layer's projection term into the
                        # accumulator so hTu can be dropped after use
                        r0, r1 = b * 128, (b + 1) * 128
                        pf = ppy.tile([128, D_LAB], F32, tag="pf",
                                      name=f"pf_{rep}_{layer}_{b}")
                        nc.tensor.matmul(
                            pf[:], h_blk,
                            wp_s[:, layer * D_LAB:(layer + 1) * D_LAB],
                            start=True, stop=(layer != 0))
                        if layer == 0:
                            nc.tensor.matmul(pf[:], sqdeg_s[:, r0:r1],
                                             bp_s[:], start=False, stop=True)
                        acc_sl = proj_acc[:, b * D_LAB:(b + 1) * D_LAB]
                        if layer == 0:
                            nc.vector.tensor_copy(acc_sl, pf[:])
                        else:
                            nc.vector.tensor_add(acc_sl, acc_sl, pf[:])

            def emit_projection_part(rep, qq):
                """out[t, l] = dinv[t]*(sum_i hTu_i^T Wp_i + sqrt(deg) bp)."""
                for j in range(PBLK[qq]):
                    b = PSTART[qq] + j
                    r0, r1 = b * 128, (b + 1) * 128
                    pf = ppy.tile([128, D_LAB], F32, tag="pf",
                                  name=f"pf_{rep}_{b}")
                    nc.tensor.matmul(pf[:], sqdeg_s[:, r0:r1], bp_s[:],
                                     start=True, stop=False)
                    for i in range(3):
                        nc.tensor.matmul(
                            pf[:], hT[(i, qq)][:, j * 128:(j + 1) * 128],
                            wp_s[:, i * D_LAB:(i + 1) * D_LAB],
                            start=False, stop=(i == 2))
                    fo = wpool.tile([128, D_LAB], F32, tag="fo")
                    nc.scalar.activation(fo[:], pf[:], AF.Copy,
                                         bias=0.0, scale=dinv_s[:, b:b + 1])
                    nc.sync.dma_start(out_d[r0:r1, :], fo[:])

            # part qq's hTu is complete after this aggregation group closes
            PART_READY = {(PSTART[p] + PBLK[p] - 1) // G: p for p in range(NQ)}
            do_proj = not (_SKIP_FINAL or _NLAYERS < 3)

            for rep in range(_REPEAT):
                if _FUSEPROJ and do_proj:
                    proj_acc = apool.tile([128, B * D_LAB], F32, tag="acc",
                                          name=f"acc_{rep}")
                for qq in range(NQ):
                    emit_transform_part(rep, 0, qq)
                emit_allgather(rep, 0)
                for layer in range(_NLAYERS):
                    if not _ILV and layer > 0:
                        for qq in range(NQ):
                            emit_transform_part(rep, layer, qq)
                        emit_allgather(rep, layer)
                    if _SKIP_AGG:
                        for qq in range(NQ):
                            h_q = hpool.tile([128, PROWS[qq]], BF16, tag="h",
                                             name=f"h_{rep}_{layer}_{qq}")
                            nc.vector.tensor_copy(h_q[:],
                                                  hT[("y", layer, qq)][:])
                            hT[(rep, layer, qq)] = h_q
                            if layer + 1 < _NLAYERS:
                                emit_transform_part(rep, layer + 1, qq)
                                if qq == NQ - 1:
                                    emit_allgather(rep, layer + 1)
                        continue
                    cur = [0]
                    for g in range(NG):
                        emit_agg_group(rep, layer, g, cur)
                        p = PART_READY.get(g)
                        if _ILV and p is not None:
                            # interleave next-layer transform (or projection)
                            # so its stage writes hide behind this aggregation
                            if layer + 1 < _NLAYERS:
                                emit_transform_part(rep, layer + 1, p)
                                if p == NQ - 1:
                                    emit_allgather(rep, layer + 1)
                            elif (layer == _NLAYERS - 1 and do_proj
                                  and not _FUSEPROJ):
                                emit_projection_part(rep, p)
                    assert cur[0] == len(segs)
                if not _ILV and do_proj and not _FUSEPROJ:
                    for qq in range(NQ):
                        emit_projection_part(rep, qq)
                if _FUSEPROJ and do_proj:
                    for b in range(B):
                        r0, r1 = b * 128, (b + 1) * 128
                        fo = wpool.tile([128, D_LAB], F32, tag="fo")
                        nc.scalar.activation(
                            fo[:], proj_acc[:, b * D_LAB:(b + 1) * D_LAB],
                            AF.Copy, bias=0.0, scale=dinv_s[:, b:b + 1])
                        nc.sync.dma_start(out_d[r0:r1, :], fo[:])

    nc.compile()
    return nc


_CACHE = {}


def _get_program(edge_index):
    key = hash(np.asarray(edge_index).tobytes())
    if key not in _CACHE:
        pre = _preprocess(edge_index)
        nc = _build(pre)
        _CACHE.clear()
        _CACHE[key] = (pre, nc)
    return _CACHE[key]


def prepare(feat, edge_index, W1, b1, W2, b2, W3, b3, Wp, bp):
    """Build (nc, in_maps) for the SPMD run."""
    feat = np.asarray(feat, np.float32)
    edge_index = np.asarray(edge_index, np.int32)
    W1, b1, W2, b2, W3, b3, Wp, bp = (np.asarray(a, np.float32)
                                      for a in (W1, b1, W2, b2, W3, b3, Wp, bp))
    pre, nc = _get_program(edge_index)

    w_all = np.concatenate([W1, W2, W3], axis=1).astype(ml_dtypes.bfloat16)
    b_all = np.concatenate([b1, b2, b3]).reshape(1, 3 * D).astype(np.float32)
    wp_all = np.concatenate([Wp[:D], Wp[D:2 * D], Wp[2 * D:]],
                            axis=1).astype(ml_dtypes.bfloat16)
    iota = np.broadcast_to(np.arange(128, dtype=np.float32),
                           (128, 128)).astype(ml_dtypes.bfloat16)
    ident = np.eye(128, dtype=np.float32).astype(ml_dtypes.bfloat16)

    feat_p = np.zeros((NCORES, SHARD_P, D), np.float32)
    feat_p[:, :SHARD] = feat.reshape(NCORES, SHARD, D)
    featT = np.ascontiguousarray(
        feat_p.transpose(0, 2, 1)).astype(ml_dtypes.bfloat16)

    in_maps = []
    for c in range(NCORES):
        in_maps.append({
            "featT": featT[c],
            "idx": pre["idx"][c],
            "tgt": pre["tgt"][c],
            "dinv_col": pre["dinv_col"][c],
            "dinv2_col": pre["dinv2_col"][c],
            "sqdeg_row": pre["sqdeg_row"][c],
            "w_all": w_all, "b_all": b_all,
            "wp_all": wp_all, "bp": bp.reshape(1, D_LAB).astype(np.float32),
            "iota": iota, "ident": ident,
        })
    return nc, in_maps


def kernel(**inputs):
    nc, in_maps = prepare(**inputs)
    trace = bool(int(os.environ.get("GCN_TRACE", "0")))
    res = bass_utils.run_bass_kernel_spmd(nc, in_maps,
                                          core_ids=list(range(NCORES)),
                                          trace=trace)
    global LAST_RESULTS
    LAST_RESULTS = res
    out = np.empty((N_NODES, D_LAB), np.float32)
    for c in range(NCORES):
        out[c * SHARD:(c + 1) * SHARD] = res.results[c]["out"][:SHARD]
    return out


LAST_RESULTS = None



# revision 16
# speedup vs baseline: 1.0759x; 1.0461x over previous
"""GCN (3-layer + linear head) Trainium2 Bass kernel, sharded over 8 NeuronCores.

v2 strategy (vertex partitioning per the sharding hint, all-bf16 datapath):
 - Nodes sharded contiguously: core c owns nodes [c*12500, (c+1)*12500),
   padded to 12544 = 98 blocks of 128 rows.
 - Per layer: transform y = dinv^k * (h @ W) runs directly off the
   transposed hidden state hT kept in SBUF (no PE transposes anywhere);
   y shards are AllGathered part-wise (4 partial collectives per layer,
   bf16) so the halo exchange overlaps the transform and aggregation.
 - Aggregation gathers per-edge source rows (dma_gather, 256B bf16 rows)
   and realizes the segment-sum as one-hot matmuls accumulating into
   per-target-block PSUM banks. One-hot builds are batched per segment
   with a single broadcast-AP is_equal on the vector engine.
 - Self-loop messages skip the gather entirely: they are applied as a
   diag(dinv) matmul against the local y block in SBUF.
 - The hidden state is stored unscaled (relu(dinv*(x))=dinv*relu(x));
   dinv factors fold into the next transform (dinv^2) and the final
   projection (dinv), both per-partition activation scales.
 - Host does integer index prep only: degree counts, edge sorting by
   (core, group, part, block, source), structural chunk padding shared
   across cores so all 8 run one SPMD instruction stream.
"""
import os
import sys

sys.path.insert(0, "/opt/trn_rl_repo")

import numpy as np
import ml_dtypes

_NLAYERS = int(os.environ.get("GCN_NLAYERS", "3"))
_SKIP_AGG = bool(int(os.environ.get("GCN_SKIP_AGG", "0")))
_SKIP_FINAL = bool(int(os.environ.get("GCN_SKIP_FINAL", "0")))
_NO_MM = bool(int(os.environ.get("GCN_NO_MM", "0")))
_NO_SBUILD = bool(int(os.environ.get("GCN_NO_SBUILD", "0")))
_ONECORE = bool(int(os.environ.get("GCN_ONECORE", "0")))
_REPEAT = int(os.environ.get("GCN_REPEAT", "1"))
_NAG = int(os.environ.get("GCN_NAG", "4"))       # parts per AllGather split
_NO_AG = bool(int(os.environ.get("GCN_NO_AG", "0")))
_ILV = bool(int(os.environ.get("GCN_INTERLEAVE", "0")))
_FUSEPROJ = bool(int(os.environ.get("GCN_FUSEPROJ", "1")))
_NQUEUE = int(os.environ.get("GCN_NQUEUE", "4"))
_MPOOL = int(os.environ.get("GCN_MPOOL", "14"))
_PPA = int(os.environ.get("GCN_PPA", "4"))
_SPOOL = int(os.environ.get("GCN_SPOOL", "3"))
_YPOOL = int(os.environ.get("GCN_YPOOL", "4"))

import concourse.bacc as bacc
import concourse.mybir as mybir
import concourse.tile as tile
from concourse import bass_utils
from concourse.bass import AP
from concourse.library_config import mlp

# Problem constants (hardcoded per harness contract).
N_NODES = 100000
N_EDGES = 1600000
D = 128
D_LAB = 10
NCORES = 8
SHARD = 12500
SHARD_P = 12544           # 98 blocks of 128 rows
B = SHARD_P // 128        # 98
G = 4                     # target blocks per aggregation group (psum banks)
NG = -(-B // G)           # 25 groups
PBLK = [25, 25, 25, 23]   # blocks per source part (AllGather split)
PSTART = [0, 25, 50, 75]  # first block of each part
PROWS = [p * 128 for p in PBLK]
NQ = 4                    # source parts (int16-indexable: 8*3200 = 25600)

F32 = mybir.dt.float32
BF16 = mybir.dt.bfloat16
I16 = mybir.dt.int16
AF = mybir.ActivationFunctionType
ALU = mybir.AluOpType

PAD_TGT = 200.0           # bf16-exact, outside [0,128): padded slots match nothing


QOFF = [0, 3200, 6400, 9600]  # row offset of each transform part in the stage


def _preprocess(edge_index):
    """Host-side integer/index prep. Returns per-core arrays + shared structure.

    v3: one AllGather per layer — the gathered buffer is core-major
    [core, stage_row]; gather source groups are core PAIRS (idx16 =
    (c_s % 2) * SHARD_P + stage_row <= 25087, int16-safe) with the
    dma_gather base AP offset selecting the pair."""
    src = np.asarray(edge_index[0], dtype=np.int64)
    tgt = np.asarray(edge_index[1], dtype=np.int64)

    # degree includes the self-loop (handled on-device via diag matmul)
    deg = (np.bincount(tgt, minlength=N_NODES) + 1).astype(np.float64)

    c_t = tgt // SHARD
    tl = tgt % SHARD
    blk = tl // 128
    grp = blk // G
    c_s = src // SHARD
    r = src % SHARD
    rq = np.minimum(r // 3200, 3)
    pstart = np.asarray(PSTART, np.int64)[rq]
    pblk = np.asarray(PBLK, np.int64)[rq]
    qoff = np.asarray(QOFF, np.int64)[rq]
    stage_row = qoff + (r % 128) * pblk + (r // 128 - pstart)
    q = c_s // 2  # gather source group = core pair
    qrel = (c_s % 2) * SHARD_P + stage_row

    order = np.lexsort((qrel, blk, q, grp, c_t))
    c_t, tl, blk, q, qrel = c_t[order], tl[order], blk[order], q[order], qrel[order]

    # segment = (core, g, q, blk); structural chunk count = max over cores
    seg_key = ((c_t * NG + (blk // G)) * NQ + q) * B + blk
    nseg = NCORES * NG * NQ * B
    counts = np.bincount(seg_key, minlength=nseg).reshape(NCORES, NG, NQ, B)
    C = (-(-counts // 128)).max(axis=0)  # [NG, NQ, B]

    tot_chunks = int(C.sum())
    TOT = tot_chunks * 128
    idx_all = np.zeros((NCORES, TOT), dtype=np.int16)
    tgt_all = np.full((NCORES, TOT), PAD_TGT, dtype=np.float32)

    seg_starts = np.zeros(nseg + 1, dtype=np.int64)
    np.cumsum(np.bincount(seg_key, minlength=nseg), out=seg_starts[1:])

    segs = []       # (g, q, b, nch, ci0) in emission order
    nch_gq = np.zeros((NG, NQ), dtype=np.int64)
    off = 0
    for g in range(NG):
        for qq in range(NQ):
            for b in range(g * G, min((g + 1) * G, B)):
                nch = int(C[g, qq, b])
                if nch == 0:
                    continue
                segs.append((g, qq, b, nch, off // 128))
                nch_gq[g, qq] += nch
                for c in range(NCORES):
                    k = ((c * NG + g) * NQ + qq) * B + b
                    s0, s1 = seg_starts[k], seg_starts[k + 1]
                    n = s1 - s0
                    idx_all[c, off:off + n] = qrel[s0:s1]
                    tgt_all[c, off:off + n] = (tl[s0:s1] - b * 128)
                off += nch * 128
    assert off == TOT

    idx_wrapped = np.stack([
        np.tile(a.reshape(-1, 16).T, (8, 1)) for a in idx_all])
    tgt_tiles = np.ascontiguousarray(
        tgt_all.reshape(NCORES, tot_chunks, 128).transpose(0, 2, 1)
    ).astype(ml_dtypes.bfloat16)

    deg_p = np.ones((NCORES, SHARD_P), dtype=np.float64)
    deg_p[:, :SHARD] = deg.reshape(NCORES, SHARD)
    dinv = 1.0 / np.sqrt(deg_p)
    dinv_col = np.ascontiguousarray(
        dinv.reshape(NCORES, B, 128).transpose(0, 2, 1)).astype(np.float32)
    dinv2_col = np.ascontiguousarray(
        (1.0 / deg_p).reshape(NCORES, B, 128).transpose(0, 2, 1)).astype(np.float32)
    sqdeg_row = np.sqrt(deg_p).reshape(NCORES, 1, SHARD_P).astype(np.float32)

    return dict(idx=idx_wrapped, tgt=tgt_tiles, dinv_col=dinv_col,
                dinv2_col=dinv2_col, sqdeg_row=sqdeg_row,
                segs=segs, nch_gq=nch_gq, tot_chunks=tot_chunks, TOT=TOT)


def _build(pre):
    """Build the Bass/Tile program (one SPMD NEFF for all 8 cores)."""
    zero_bias = bool(pre.get("zero_bias", False))
    TOT = pre["TOT"]
    tot_chunks = pre["tot_chunks"]
    nch_gq = pre["nch_gq"]
    segs = pre["segs"]

    nc = bacc.Bacc("TRN2", target_bir_lowering=False, debug=False,
                   num_devices=1 if _ONECORE else NCORES,
                   num_swdge_queues=_NQUEUE)

    featT_d = nc.dram_tensor("featT", [D, SHARD_P], BF16, kind="ExternalInput")
    idx_d = nc.dram_tensor("idx", [128, TOT // 16], I16, kind="ExternalInput")
    tgt_d = nc.dram_tensor("tgt", [128, tot_chunks], BF16, kind="ExternalInput")
    dinv_d = nc.dram_tensor("dinv_col", [128, B], F32, kind="ExternalInput")
    dinv2_d = nc.dram_tensor("dinv2_col", [128, B], F32, kind="ExternalInput")
    sqdeg_d = nc.dram_tensor("sqdeg_row", [1, SHARD_P], F32, kind="ExternalInput")
    w_d = nc.dram_tensor("w_all", [128, 3 * D], BF16, kind="ExternalInput")
    b_d = nc.dram_tensor("b_all", [1, 3 * D], F32, kind="ExternalInput")
    wp_d = nc.dram_tensor("wp_all", [128, 3 * D_LAB], BF16, kind="ExternalInput")
    bp_d = nc.dram_tensor("bp", [1, D_LAB], F32, kind="ExternalInput")
    iota_d = nc.dram_tensor("iota", [128, 128], BF16, kind="ExternalInput")
    ident_d = nc.dram_tensor("ident", [128, 128], BF16, kind="ExternalInput")

    out_d = nc.dram_tensor("out", [SHARD_P, D_LAB], F32, kind="ExternalOutput")

    with tile.TileContext(nc) as tc:
        with (
            tc.tile_pool(name="const", bufs=1) as cpool,
            tc.tile_pool(name="work", bufs=3) as wpool,
            tc.tile_pool(name="hstate", bufs=(2 if _FUSEPROJ else 3) * NQ) as hpool,
            tc.tile_pool(name="ystate", bufs=_YPOOL) as ypool,
            tc.tile_pool(name="projacc", bufs=2) as apool,
            tc.tile_pool(name="sbuild", bufs=_SPOOL) as spool,
            tc.tile_pool(name="mtiles", bufs=_MPOOL) as mpool,
            tc.tile_pool(name="itiles", bufs=3) as ipool,
            tc.tile_pool(name="psum_a", bufs=_PPA, space="PSUM") as ppa,
            tc.tile_pool(name="psum_y", bufs=2, space="PSUM") as ppy,
            tc.tile_pool(name="dram", bufs=1, space="DRAM") as dpool,
        ):
            nc.gpsimd.load_library(mlp)

            # ---- constants ----
            tgt_s = cpool.tile([128, tot_chunks], BF16)
            iota_s = cpool.tile([128, 128], BF16)
            ident_s = cpool.tile([128, 128], BF16)
            w_s = cpool.tile([128, 3 * D], BF16)
            b_s = cpool.tile([1, 3 * D], F32)
            wp_s = cpool.tile([128, 3 * D_LAB], BF16)
            bp_s = cpool.tile([1, D_LAB], F32)
            dinv_s = cpool.tile([128, B], F32)
            dinv2_s = cpool.tile([128, B], F32)
            sqdeg_s = None
            if not zero_bias:
                sqdeg_s = cpool.tile([1, SHARD_P], F32)
                nc.sync.dma_start(sqdeg_s[:], sqdeg_d[:])

            nc.sync.dma_start(tgt_s[:], tgt_d[:])
            nc.sync.dma_start(iota_s[:], iota_d[:])
            nc.sync.dma_start(ident_s[:], ident_d[:])
            nc.sync.dma_start(w_s[:], w_d[:])
            nc.sync.dma_start(b_s[:], b_d[:])
            nc.sync.dma_start(wp_s[:], wp_d[:])
            nc.sync.dma_start(bp_s[:], bp_d[:])
            nc.sync.dma_start(dinv_s[:], dinv_d[:])
            nc.sync.dma_start(dinv2_s[:], dinv2_d[:])

            # ---- internal DRAM: one AllGather landing buffer per (rep,layer)
            y_fulls = {}
            y_stage = {}
            for rep in range(_REPEAT):
                for layer in range(_NLAYERS):
                    y_fulls[(rep, layer)] = dpool.tile(
                        [NCORES * SHARD_P, D], BF16,
                        addr_space=os.environ.get("GCN_YPART_SPACE",
                                                  "Shared"),
                        name=f"y_full_{rep}_{layer}")
                    y_stage[(rep, layer)] = dpool.tile(
                        [SHARD_P, D], BF16,
                        name=f"y_stage_{rep}_{layer}")

            hT = {}          # (layer, qq) -> SBUF tile [128 f, PROWS[q] cols]
            qrr = 0          # gather queue round-robin

            def seg_onehot(st_t, ci0, nch):
                """st[:, k*128+j] = (iota[:, j] == tgt[:, ci0+k]), batched."""
                o = AP(st_t[:, :].tensor, st_t[:, :].offset,
                       [st_t[:, :].ap[0], [128, nch], [1, 128]])
                i1 = AP(iota_s[:, :].tensor, iota_s[:, :].offset,
                        [iota_s[:, :].ap[0], [0, nch], [1, 128]])
                t2 = tgt_s[:, ci0:ci0 + nch]
                i2 = AP(t2.tensor, t2.offset, [t2.ap[0], [1, nch], [0, 128]])
                nc.vector.tensor_tensor(o, i1, i2, ALU.is_equal)

            def emit_transform_part(rep, layer, qq):
                """y = dinv^k * (hT_prev^T @ W) for part qq -> stage rows."""
                wl = w_s[:, layer * D:(layer + 1) * D]
                y_q = ypool.tile([128, PROWS[qq]], BF16, tag="ysb",
                                 name=f"y_{rep}_{layer}_{qq}")
                for j in range(PBLK[qq]):
                    b = PSTART[qq] + j
                    if layer == 0:
                        fb = wpool.tile([128, 128], BF16, tag="fb")
                        nc.sync.dma_start(
                            fb[:], featT_d[:, b * 128:(b + 1) * 128])
                        stat = fb[:]
                    else:
                        stat = hT[(rep, layer - 1, qq)][:, j * 128:(j + 1) * 128]
                    yp = ppy.tile([128, 128], F32, tag="ty")
                    nc.tensor.matmul(yp[:], stat, wl, start=True, stop=True)
                    scale = (dinv_s if layer == 0 else dinv2_s)[:, b:b + 1]
                    nc.scalar.activation(y_q[:, j * 128:(j + 1) * 128],
                                         yp[:], AF.Copy,
                                         bias=0.0, scale=scale)
                stage = y_stage[(rep, layer)]
                nc.sync.dma_start(
                    stage[QOFF[qq]:QOFF[qq] + PROWS[qq], :], y_q[:])
                hT[("y", layer, qq)] = y_q

            def emit_allgather(rep, layer):
                if _ONECORE:
                    nc.sync.dma_start(
                        y_fulls[(rep, layer)][0:SHARD_P, :],
                        y_stage[(rep, layer)][:])
                    return
                if _NO_AG:
                    return
                nc.gpsimd.collective_compute(
                    "AllGather", ALU.bypass,
                    replica_groups=[list(range(NCORES))],
                    ins=[y_stage[(rep, layer)].opt()],
                    outs=[y_fulls[(rep, layer)].opt()],
                )

            def emit_agg_group(rep, layer, g, cur):
                nonlocal qrr
                bl = b_s[:, layer * D:(layer + 1) * D]
                blocks = list(range(g * G, min((g + 1) * G, B)))
                psums = {}
                opened = set()
                for b in blocks:
                    ps = ppa.tile([128, 128], F32, tag="agg",
                                  name=f"ps_{rep}_{layer}_{b}")
                    psums[b] = ps
                    if not zero_bias:
                        # open the accumulation group with the bias rank-1
                        r0, r1 = b * 128, (b + 1) * 128
                        nc.tensor.matmul(ps[:], bl, sqdeg_s[:, r0:r1],
                                         start=True, stop=False)
                        opened.add(b)
                nch_g = int(nch_gq[g].sum())
                g16_0 = segs[cur[0]][4] * 8
                idx_g = ipool.tile([128, nch_g * 8], I16, tag="idxs",
                                   name=f"ix_{rep}_{layer}_{g}")
                nc.sync.dma_start(idx_g[:], idx_d[:, g16_0:g16_0 + nch_g * 8])
                y_full = y_fulls[(rep, layer)]
                for qq in range(NQ):
                    nch = int(nch_gq[g, qq])
                    if nch == 0:
                        continue
                    ci0 = segs[cur[0]][4]
                    n_idx = nch * 128
                    n16 = n_idx // 16
                    off16 = ci0 * 8 - g16_0
                    mt = mpool.tile([128, nch, 128], BF16, tag="m",
                                    name=f"m_{rep}_{layer}_{g}_{qq}")
                    src = y_full[qq * 2 * SHARD_P:(qq + 1) * 2 * SHARD_P, :]
                    nc.gpsimd.dma_gather(
                        mt[:], src, idx_g[:, off16:off16 + n16],
                        n_idx, n_idx, D, single_packet=False,
                        queue_num=qrr % _NQUEUE)
                    qrr += 1
                    if not _NO_SBUILD:
                        # one batched is_equal for all chunks of this (g,q)
                        st_t = spool.tile([128, nch * 128], BF16, tag="s",
                                          name=f"s_{rep}_{layer}_{g}_{qq}")
                        seg_onehot(st_t, ci0, nch)
                    k = 0
                    while k < nch:
                        g2, q2, b2, nck, ci = segs[cur[0]]
                        assert g2 == g and q2 == qq and ci == ci0 + k
                        if not _NO_MM:
                            for u in range(nck):
                                first = b2 not in opened
                                opened.add(b2)
                                nc.tensor.matmul(
                                    psums[b2][:], mt[:, k + u, :],
                                    st_t[:, (k + u) * 128:(k + u + 1) * 128],
                                    start=first, stop=False)
                        k += nck
                        cur[0] += 1
                # close each block: self-loop identity matmul
                # (h = dinv*hTu, so the self message dinv^2*(hW) is just y)
                for b in blocks:
                    pq = min(b // 25, 3)
                    pj = b - PSTART[pq]
                    y_q = hT[("y", layer, pq)]
                    nc.tensor.matmul(
                        psums[b][:], y_q[:, pj * 128:(pj + 1) * 128],
                        ident_s[:], start=(b not in opened), stop=True)
                for b in blocks:
                    pq = min(b // 25, 3)
                    pj = b - PSTART[pq]
                    if (rep, layer, pq) not in hT:
                        hT[(rep, layer, pq)] = hpool.tile(
                            [128, PROWS[pq]], BF16, tag="h",
                            name=f"h_{rep}_{layer}_{pq}")
                    func = AF.Relu if layer < _NLAYERS - 1 else AF.Copy
                    h_blk = hT[(rep, layer, pq)][:, pj * 128:(pj + 1) * 128]
                    nc.scalar.activation(h_blk, psums[b][:], func)
                    if _FUSEPROJ and do_proj:
                        # fold this layer's projection term into the
                        # accumulator so hTu can be dropped after use
                        r0, r1 = b * 128, (b + 1) * 128
                        pf = ppy.tile([128, D_LAB], F32, tag="pf",
                                      name=f"pf_{rep}_{layer}_{b}")
                        nc.tensor.matmul(
                            pf[:], h_blk,
                            wp_s[:, layer * D_LAB:(layer + 1) * D_LAB],
                            start=True, stop=(layer != 0 or zero_bias))
                        if layer == 0 and not zero_bias:
                            r0, r1 = b * 128, (b + 1) * 128
                            nc.tensor.matmul(pf[:], sqdeg_s[:, r0:r1],
                                             bp_s[:], start=False, stop=True)
                        acc_sl = proj_acc[:, b * D_LAB:(b + 1) * D_LAB]
                        if layer == 0:
                            nc.vector.tensor_copy(acc_sl, pf[:])
                        else:
                            nc.vector.tensor_add(acc_sl, acc_sl, pf[:])

            def emit_projection_part(rep, qq):
                """out[t, l] = dinv[t]*(sum_i hTu_i^T Wp_i + sqrt(deg) bp)."""
                for j in range(PBLK[qq]):
                    b = PSTART[qq] + j
                    r0, r1 = b * 128, (b + 1) * 128
                    pf = ppy.tile([128, D_LAB], F32, tag="pf",
                                  name=f"pf_{rep}_{b}")
                    r0, r1 = b * 128, (b + 1) * 128
                    if not zero_bias:
                        nc.tensor.matmul(pf[:], sqdeg_s[:, r0:r1], bp_s[:],
                                         start=True, stop=False)
                    for i in range(3):
                        nc.tensor.matmul(
                            pf[:], hT[(i, qq)][:, j * 128:(j + 1) * 128],
                            wp_s[:, i * D_LAB:(i + 1) * D_LAB],
                            start=(i == 0 and zero_bias), stop=(i == 2))
                    fo = wpool.tile([128, D_LAB], F32, tag="fo")
                    nc.scalar.activation(fo[:], pf[:], AF.Copy,
                                         bias=0.0, scale=dinv_s[:, b:b + 1])
                    nc.sync.dma_start(out_d[r0:r1, :], fo[:])

            # part qq's hTu is complete after this aggregation group closes
            PART_READY = {(PSTART[p] + PBLK[p] - 1) // G: p for p in range(NQ)}
            do_proj = not (_SKIP_FINAL or _NLAYERS < 3)

            for rep in range(_REPEAT):
                if _FUSEPROJ and do_proj:
                    proj_acc = apool.tile([128, B * D_LAB], F32, tag="acc",
                                          name=f"acc_{rep}")
                for qq in range(NQ):
                    emit_transform_part(rep, 0, qq)
                emit_allgather(rep, 0)
                for layer in range(_NLAYERS):
                    if not _ILV and layer > 0:
                        for qq in range(NQ):
                            emit_transform_part(rep, layer, qq)
                        emit_allgather(rep, layer)
                    if _SKIP_AGG:
                        for qq in range(NQ):
                            h_q = hpool.tile([128, PROWS[qq]], BF16, tag="h",
                                             name=f"h_{rep}_{layer}_{qq}")
                            nc.vector.tensor_copy(h_q[:],
                                                  hT[("y", layer, qq)][:])
                            hT[(rep, layer, qq)] = h_q
                            if layer + 1 < _NLAYERS:
                                emit_transform_part(rep, layer + 1, qq)
                                if qq == NQ - 1:
                                    emit_allgather(rep, layer + 1)
                        continue
                    cur = [0]
                    for g in range(NG):
                        emit_agg_group(rep, layer, g, cur)
                        p = PART_READY.get(g)
                        if _ILV and p is not None:
                            # interleave next-layer transform (or projection)
                            # so its stage writes hide behind this aggregation
                            if layer + 1 < _NLAYERS:
                                emit_transform_part(rep, layer + 1, p)
                                if p == NQ - 1:
                                    emit_allgather(rep, layer + 1)
                            elif (layer == _NLAYERS - 1 and do_proj
                                  and not _FUSEPROJ):
                                emit_projection_part(rep, p)
                    assert cur[0] == len(segs)
                if not _ILV and do_proj and not _FUSEPROJ:
                    for qq in range(NQ):
                        emit_projection_part(rep, qq)
                if _FUSEPROJ and do_proj:
                    for b in range(B):
                        r0, r1 = b * 128, (b + 1) * 128
                        fo = wpool.tile([128, D_LAB], F32, tag="fo")
                        nc.scalar.activation(
                            fo[:], proj_acc[:, b * D_LAB:(b + 1) * D_LAB],
                            AF.Copy, bias=0.0, scale=dinv_s[:, b:b + 1])
                        nc.sync.dma_start(out_d[r0:r1, :], fo[:])

    nc.compile()
    return nc


_CACHE = {}


def _get_program(edge_index, zero_bias):
    key = (hash(np.asarray(edge_index).tobytes()), zero_bias)
    if key not in _CACHE:
        pre = _preprocess(edge_index)
        pre["zero_bias"] = zero_bias
        nc = _build(pre)
        _CACHE.clear()
        _CACHE[key] = (pre, nc)
    return _CACHE[key]


def prepare(feat, edge_index, W1, b1, W2, b2, W3, b3, Wp, bp):
    """Build (nc, in_maps) for the SPMD run."""
    feat = np.asarray(feat, np.float32)
    edge_index = np.asarray(edge_index, np.int32)
    W1, b1, W2, b2, W3, b3, Wp, bp = (np.asarray(a, np.float32)
                                      for a in (W1, b1, W2, b2, W3, b3, Wp, bp))
    zero_bias = not (np.any(b1) or np.any(b2) or np.any(b3) or np.any(bp))
    pre, nc = _get_program(edge_index, zero_bias)

    w_all = np.concatenate([W1, W2, W3], axis=1).astype(ml_dtypes.bfloat16)
    b_all = np.concatenate([b1, b2, b3]).reshape(1, 3 * D).astype(np.float32)
    wp_all = np.concatenate([Wp[:D], Wp[D:2 * D], Wp[2 * D:]],
                            axis=1).astype(ml_dtypes.bfloat16)
    iota = np.broadcast_to(np.arange(128, dtype=np.float32),
                           (128, 128)).astype(ml_dtypes.bfloat16)
    ident = np.eye(128, dtype=np.float32).astype(ml_dtypes.bfloat16)

    feat_p = np.zeros((NCORES, SHARD_P, D), np.float32)
    feat_p[:, :SHARD] = feat.reshape(NCORES, SHARD, D)
    featT = np.ascontiguousarray(
        feat_p.transpose(0, 2, 1)).astype(ml_dtypes.bfloat16)

    in_maps = []
    for c in range(NCORES):
        in_maps.append({
            "featT": featT[c],
            "idx": pre["idx"][c],
            "tgt": pre["tgt"][c],
            "dinv_col": pre["dinv_col"][c],
            "dinv2_col": pre["dinv2_col"][c],
            "sqdeg_row": pre["sqdeg_row"][c],
            "w_all": w_all, "b_all": b_all,
            "wp_all": wp_all, "bp": bp.reshape(1, D_LAB).astype(np.float32),
            "iota": iota, "ident": ident,
        })
    return nc, in_maps


def kernel(**inputs):
    nc, in_maps = prepare(**inputs)
    trace = bool(int(os.environ.get("GCN_TRACE", "0")))
    res = bass_utils.run_bass_kernel_spmd(nc, in_maps,
                                          core_ids=list(range(NCORES)),
                                          trace=trace)
    global LAST_RESULTS
    LAST_RESULTS = res
    out = np.empty((N_NODES, D_LAB), np.float32)
    for c in range(NCORES):
        out[c * SHARD:(c + 1) * SHARD] = res.results[c]["out"][:SHARD]
    return out


LAST_RESULTS = None



# revision 18
# speedup vs baseline: 1.3929x; 1.2947x over previous
"""GCN (3-layer + linear head) Trainium2 Bass kernel, sharded over 8 NeuronCores.

v2 strategy (vertex partitioning per the sharding hint, all-bf16 datapath):
 - Nodes sharded contiguously: core c owns nodes [c*12500, (c+1)*12500),
   padded to 12544 = 98 blocks of 128 rows.
 - Per layer: transform y = dinv^k * (h @ W) runs directly off the
   transposed hidden state hT kept in SBUF (no PE transposes anywhere);
   y shards are AllGathered part-wise (4 partial collectives per layer,
   bf16) so the halo exchange overlaps the transform and aggregation.
 - Aggregation gathers per-edge source rows (dma_gather, 256B bf16 rows)
   and realizes the segment-sum as one-hot matmuls accumulating into
   per-target-block PSUM banks. One-hot builds are batched per segment
   with a single broadcast-AP is_equal on the vector engine.
 - Self-loop messages skip the gather entirely: they are applied as a
   diag(dinv) matmul against the local y block in SBUF.
 - The hidden state is stored unscaled (relu(dinv*(x))=dinv*relu(x));
   dinv factors fold into the next transform (dinv^2) and the final
   projection (dinv), both per-partition activation scales.
 - Host does integer index prep only: degree counts, edge sorting by
   (core, group, part, block, source), structural chunk padding shared
   across cores so all 8 run one SPMD instruction stream.
"""
import os
import sys

sys.path.insert(0, "/opt/trn_rl_repo")

import numpy as np
import ml_dtypes

_NLAYERS = int(os.environ.get("GCN_NLAYERS", "3"))
_SKIP_AGG = bool(int(os.environ.get("GCN_SKIP_AGG", "0")))
_SKIP_FINAL = bool(int(os.environ.get("GCN_SKIP_FINAL", "0")))
_NO_MM = bool(int(os.environ.get("GCN_NO_MM", "0")))
_NO_SBUILD = bool(int(os.environ.get("GCN_NO_SBUILD", "0")))
_ONECORE = bool(int(os.environ.get("GCN_ONECORE", "0")))
_REPEAT = int(os.environ.get("GCN_REPEAT", "1"))
_NAG = int(os.environ.get("GCN_NAG", "4"))       # parts per AllGather split
_NO_AG = bool(int(os.environ.get("GCN_NO_AG", "0")))
_ILV = bool(int(os.environ.get("GCN_INTERLEAVE", "0")))
_FUSEPROJ = bool(int(os.environ.get("GCN_FUSEPROJ", "1")))
_NQUEUE = int(os.environ.get("GCN_NQUEUE", "4"))
_MPOOL = int(os.environ.get("GCN_MPOOL", "14"))
_PPA = int(os.environ.get("GCN_PPA", "4"))
_SPOOL = int(os.environ.get("GCN_SPOOL", "3"))
_YPOOL = int(os.environ.get("GCN_YPOOL", "4"))

import concourse.bacc as bacc
import concourse.mybir as mybir
import concourse.tile as tile
from concourse import bass_utils
from concourse.bass import AP
from concourse.library_config import mlp

# Problem constants (hardcoded per harness contract).
N_NODES = 100000
N_EDGES = 1600000
D = 128
D_LAB = 10
NCORES = 8
SHARD = 12500
B = 104                   # target blocks per core (128 rows each, ~120 used)
SHARD_P = B * 128         # 13312 padded rows per core
G = 4                     # target blocks per aggregation group (psum banks)
NG = -(-B // G)           # 26 groups
BPP = B // 4              # blocks per transform part (26)
PBLK = [BPP] * 4
PSTART = [0, BPP, 2 * BPP, 3 * BPP]
PROWS = [p * 128 for p in PBLK]
NQ = 4                    # gather source groups (core pairs)

F32 = mybir.dt.float32
BF16 = mybir.dt.bfloat16
I16 = mybir.dt.int16
AF = mybir.ActivationFunctionType
ALU = mybir.AluOpType

PAD_TGT = 200.0           # bf16-exact, outside [0,128): padded slots match nothing


QOFF = [0, PROWS[0], 2 * PROWS[0], 3 * PROWS[0]]  # part row offsets in stage


def _node_layout(src, tgt):
    """Per-core node->slot permutation: stratified deal + repair so every
    (source-pair, target-block) cell holds <= 512 edges (C=4 chunks), which
    eliminates nearly all structural SPMD padding."""
    pair = (src // SHARD) // 2
    c_t_all = tgt // SHARD
    newpos = np.zeros((NCORES, SHARD), np.int64)
    for c in range(NCORES):
        m = c_t_all == c
        tl0 = tgt[m] % SHARD
        v = np.zeros((SHARD, 4), np.int64)
        np.add.at(v, (tl0, pair[m]), 1)
        order = np.lexsort((v[:, 3], v[:, 2], v[:, 1], v[:, 0]))
        node_blk = np.zeros(SHARD, np.int64)
        node_blk[order] = np.arange(SHARD) % B
        for _ in range(3):
            bs = np.stack([np.bincount(node_blk, weights=v[:, qq],
                                       minlength=B)
                           for qq in range(4)], axis=1).astype(np.int64)
            cap = np.bincount(node_blk, minlength=B)
            over = np.argwhere(bs > 512)
            if len(over) == 0:
                break
            for b, qq in over:
                need = bs[b, qq] - 512
                nodes = np.where(node_blk == b)[0]
                cand = nodes[np.argsort(-(2 * v[nodes, qq]
                                          - v[nodes].sum(1)))]
                for n in cand:
                    if need <= 0 or v[n, qq] == 0:
                        break
                    slack = 512 - bs[:, qq]
                    slack[cap >= 126] = -1 << 30
                    b2 = int(np.argmax(slack))
                    if slack[b2] < v[n, qq]:
                        break
                    node_blk[n] = b2
                    bs[b] -= v[n]
                    bs[b2] += v[n]
                    cap[b] -= 1
                    cap[b2] += 1
                    need -= v[n, qq]
        o2 = np.argsort(node_blk, kind="stable")
        starts = np.searchsorted(node_blk[o2], np.arange(B))
        ranks = np.arange(SHARD) - starts[node_blk[o2]]
        within = np.zeros(SHARD, np.int64)
        within[o2] = ranks
        assert within.max() < 128
        newpos[c] = node_blk * 128 + within
    return newpos


def _stage_row_of(slot):
    """Stage row of a node at padded local slot (transposed part layout)."""
    rq = slot // PROWS[0]
    qoff = np.asarray(QOFF, np.int64)[rq]
    pstart = np.asarray(PSTART, np.int64)[rq]
    return qoff + (slot % 128) * BPP + (slot // 128 - pstart)


def _preprocess(edge_index):
    """Host-side integer/index prep. Returns per-core arrays + shared structure.

    One AllGather per layer — the gathered buffer is core-major
    [core, stage_row]; gather source groups are core PAIRS (idx16 =
    (c_s % 2) * SHARD_P + stage_row <= 26623, int16-safe) with the
    dma_gather base AP offset selecting the pair."""
    src = np.asarray(edge_index[0], dtype=np.int64)
    tgt = np.asarray(edge_index[1], dtype=np.int64)

    newpos = _node_layout(src, tgt)

    # degree includes the self-loop (handled on-device via diag matmul)
    deg = (np.bincount(tgt, minlength=N_NODES) + 1).astype(np.float64)

    c_t = tgt // SHARD
    tl = newpos[c_t, tgt % SHARD]
    blk = tl // 128
    grp = blk // G
    c_s = src // SHARD
    r = newpos[c_s, src % SHARD]
    stage_row = _stage_row_of(r)
    q = c_s // 2  # gather source group = core pair
    qrel = (c_s % 2) * SHARD_P + stage_row

    order = np.lexsort((qrel, blk, q, grp, c_t))
    c_t, tl, blk, q, qrel = c_t[order], tl[order], blk[order], q[order], qrel[order]

    # segment = (core, g, q, blk); structural chunk count = max over cores
    seg_key = ((c_t * NG + (blk // G)) * NQ + q) * B + blk
    nseg = NCORES * NG * NQ * B
    counts = np.bincount(seg_key, minlength=nseg).reshape(NCORES, NG, NQ, B)
    C = (-(-counts // 128)).max(axis=0)  # [NG, NQ, B]

    tot_chunks = int(C.sum())
    TOT = tot_chunks * 128
    idx_all = np.zeros((NCORES, TOT), dtype=np.int16)
    tgt_all = np.full((NCORES, TOT), PAD_TGT, dtype=np.float32)

    seg_starts = np.zeros(nseg + 1, dtype=np.int64)
    np.cumsum(np.bincount(seg_key, minlength=nseg), out=seg_starts[1:])

    segs = []       # (g, q, b, nch, ci0) in emission order
    nch_gq = np.zeros((NG, NQ), dtype=np.int64)
    off = 0
    for g in range(NG):
        for qq in range(NQ):
            for b in range(g * G, min((g + 1) * G, B)):
                nch = int(C[g, qq, b])
                if nch == 0:
                    continue
                segs.append((g, qq, b, nch, off // 128))
                nch_gq[g, qq] += nch
                for c in range(NCORES):
                    k = ((c * NG + g) * NQ + qq) * B + b
                    s0, s1 = seg_starts[k], seg_starts[k + 1]
                    n = s1 - s0
                    idx_all[c, off:off + n] = qrel[s0:s1]
                    tgt_all[c, off:off + n] = (tl[s0:s1] - b * 128)
                off += nch * 128
    assert off == TOT

    idx_wrapped = np.stack([
        np.tile(a.reshape(-1, 16).T, (8, 1)) for a in idx_all])
    tgt_tiles = np.ascontiguousarray(
        tgt_all.reshape(NCORES, tot_chunks, 128).transpose(0, 2, 1)
    ).astype(ml_dtypes.bfloat16)

    deg_p = np.ones((NCORES, SHARD_P), dtype=np.float64)
    rows = np.repeat(np.arange(NCORES), SHARD)
    deg_p[rows, newpos.ravel()] = deg
    dinv = 1.0 / np.sqrt(deg_p)
    dinv_col = np.ascontiguousarray(
        dinv.reshape(NCORES, B, 128).transpose(0, 2, 1)).astype(np.float32)
    dinv2_col = np.ascontiguousarray(
        (1.0 / deg_p).reshape(NCORES, B, 128).transpose(0, 2, 1)).astype(np.float32)
    sqdeg_row = np.sqrt(deg_p).reshape(NCORES, 1, SHARD_P).astype(np.float32)

    return dict(idx=idx_wrapped, tgt=tgt_tiles, dinv_col=dinv_col,
                dinv2_col=dinv2_col, sqdeg_row=sqdeg_row,
                segs=segs, nch_gq=nch_gq, tot_chunks=tot_chunks, TOT=TOT,
                newpos=newpos)


def _build(pre):
    """Build the Bass/Tile program (one SPMD NEFF for all 8 cores)."""
    zero_bias = bool(pre.get("zero_bias", False))
    TOT = pre["TOT"]
    tot_chunks = pre["tot_chunks"]
    nch_gq = pre["nch_gq"]
    segs = pre["segs"]

    nc = bacc.Bacc("TRN2", target_bir_lowering=False, debug=False,
                   num_devices=1 if _ONECORE else NCORES,
                   num_swdge_queues=_NQUEUE)

    featT_d = nc.dram_tensor("featT", [D, SHARD_P], BF16, kind="ExternalInput")
    idx_d = nc.dram_tensor("idx", [128, TOT // 16], I16, kind="ExternalInput")
    tgt_d = nc.dram_tensor("tgt", [128, tot_chunks], BF16, kind="ExternalInput")
    dinv_d = nc.dram_tensor("dinv_col", [128, B], F32, kind="ExternalInput")
    dinv2_d = nc.dram_tensor("dinv2_col", [128, B], F32, kind="ExternalInput")
    sqdeg_d = nc.dram_tensor("sqdeg_row", [1, SHARD_P], F32, kind="ExternalInput")
    w_d = nc.dram_tensor("w_all", [128, 3 * D], BF16, kind="ExternalInput")
    b_d = nc.dram_tensor("b_all", [1, 3 * D], F32, kind="ExternalInput")
    wp_d = nc.dram_tensor("wp_all", [128, 3 * D_LAB], BF16, kind="ExternalInput")
    bp_d = nc.dram_tensor("bp", [1, D_LAB], F32, kind="ExternalInput")
    iota_d = nc.dram_tensor("iota", [128, 128], BF16, kind="ExternalInput")
    ident_d = nc.dram_tensor("ident", [128, 128], BF16, kind="ExternalInput")

    out_d = nc.dram_tensor("out", [SHARD_P, D_LAB], F32, kind="ExternalOutput")

    with tile.TileContext(nc) as tc:
        with (
            tc.tile_pool(name="const", bufs=1) as cpool,
            tc.tile_pool(name="work", bufs=3) as wpool,
            tc.tile_pool(name="hstate", bufs=(2 if _FUSEPROJ else 3) * NQ) as hpool,
            tc.tile_pool(name="ystate", bufs=_YPOOL) as ypool,
            tc.tile_pool(name="projacc", bufs=2) as apool,
            tc.tile_pool(name="sbuild", bufs=_SPOOL) as spool,
            tc.tile_pool(name="mtiles", bufs=_MPOOL) as mpool,
            tc.tile_pool(name="itiles", bufs=3) as ipool,
            tc.tile_pool(name="psum_a", bufs=_PPA, space="PSUM") as ppa,
            tc.tile_pool(name="psum_y", bufs=2, space="PSUM") as ppy,
            tc.tile_pool(name="dram", bufs=1, space="DRAM") as dpool,
        ):
            nc.gpsimd.load_library(mlp)

            # ---- constants ----
            tgt_s = cpool.tile([128, tot_chunks], BF16)
            iota_s = cpool.tile([128, 128], BF16)
            ident_s = cpool.tile([128, 128], BF16)
            w_s = cpool.tile([128, 3 * D], BF16)
            b_s = cpool.tile([1, 3 * D], F32)
            wp_s = cpool.tile([128, 3 * D_LAB], BF16)
            bp_s = cpool.tile([1, D_LAB], F32)
            dinv_s = cpool.tile([128, B], F32)
            dinv2_s = cpool.tile([128, B], F32)
            sqdeg_s = None
            if not zero_bias:
                sqdeg_s = cpool.tile([1, SHARD_P], F32)
                nc.sync.dma_start(sqdeg_s[:], sqdeg_d[:])

            nc.sync.dma_start(tgt_s[:], tgt_d[:])
            nc.sync.dma_start(iota_s[:], iota_d[:])
            nc.sync.dma_start(ident_s[:], ident_d[:])
            nc.sync.dma_start(w_s[:], w_d[:])
            nc.sync.dma_start(b_s[:], b_d[:])
            nc.sync.dma_start(wp_s[:], wp_d[:])
            nc.sync.dma_start(bp_s[:], bp_d[:])
            nc.sync.dma_start(dinv_s[:], dinv_d[:])
            nc.sync.dma_start(dinv2_s[:], dinv2_d[:])

            # ---- internal DRAM: one AllGather landing buffer per (rep,layer)
            y_fulls = {}
            y_stage = {}
            for rep in range(_REPEAT):
                for layer in range(_NLAYERS):
                    y_fulls[(rep, layer)] = dpool.tile(
                        [NCORES * SHARD_P, D], BF16,
                        addr_space=os.environ.get("GCN_YPART_SPACE",
                                                  "Shared"),
                        name=f"y_full_{rep}_{layer}")
                    y_stage[(rep, layer)] = dpool.tile(
                        [SHARD_P, D], BF16,
                        name=f"y_stage_{rep}_{layer}")

            hT = {}          # (layer, qq) -> SBUF tile [128 f, PROWS[q] cols]
            qrr = 0          # gather queue round-robin

            def seg_onehot(st_t, ci0, nch):
                """st[:, k*128+j] = (iota[:, j] == tgt[:, ci0+k]), batched."""
                o = AP(st_t[:, :].tensor, st_t[:, :].offset,
                       [st_t[:, :].ap[0], [128, nch], [1, 128]])
                i1 = AP(iota_s[:, :].tensor, iota_s[:, :].offset,
                        [iota_s[:, :].ap[0], [0, nch], [1, 128]])
                t2 = tgt_s[:, ci0:ci0 + nch]
                i2 = AP(t2.tensor, t2.offset, [t2.ap[0], [1, nch], [0, 128]])
                nc.vector.tensor_tensor(o, i1, i2, ALU.is_equal)

            def emit_transform_part(rep, layer, qq):
                """y = dinv^k * (hT_prev^T @ W) for part qq -> stage rows."""
                wl = w_s[:, layer * D:(layer + 1) * D]
                y_q = ypool.tile([128, PROWS[qq]], BF16, tag="ysb",
                                 name=f"y_{rep}_{layer}_{qq}")
                for j in range(PBLK[qq]):
                    b = PSTART[qq] + j
                    if layer == 0:
                        fb = wpool.tile([128, 128], BF16, tag="fb")
                        nc.sync.dma_start(
                            fb[:], featT_d[:, b * 128:(b + 1) * 128])
                        stat = fb[:]
                    else:
                        stat = hT[(rep, layer - 1, qq)][:, j * 128:(j + 1) * 128]
                    yp = ppy.tile([128, 128], F32, tag="ty")
                    nc.tensor.matmul(yp[:], stat, wl, start=True, stop=True)
                    scale = (dinv_s if layer == 0 else dinv2_s)[:, b:b + 1]
                    nc.scalar.activation(y_q[:, j * 128:(j + 1) * 128],
                                         yp[:], AF.Copy,
                                         bias=0.0, scale=scale)
                stage = y_stage[(rep, layer)]
                nc.sync.dma_start(
                    stage[QOFF[qq]:QOFF[qq] + PROWS[qq], :], y_q[:])
                hT[("y", layer, qq)] = y_q

            def emit_allgather(rep, layer):
                if _ONECORE:
                    nc.sync.dma_start(
                        y_fulls[(rep, layer)][0:SHARD_P, :],
                        y_stage[(rep, layer)][:])
                    return
                if _NO_AG:
                    return
                nc.gpsimd.collective_compute(
                    "AllGather", ALU.bypass,
                    replica_groups=[list(range(NCORES))],
                    ins=[y_stage[(rep, layer)].opt()],
                    outs=[y_fulls[(rep, layer)].opt()],
                )

            def emit_agg_group(rep, layer, g, cur):
                nonlocal qrr
                bl = b_s[:, layer * D:(layer + 1) * D]
                blocks = list(range(g * G, min((g + 1) * G, B)))
                psums = {}
                opened = set()
                for b in blocks:
                    ps = ppa.tile([128, 128], F32, tag="agg",
                                  name=f"ps_{rep}_{layer}_{b}")
                    psums[b] = ps
                    if not zero_bias:
                        # open the accumulation group with the bias rank-1
                        r0, r1 = b * 128, (b + 1) * 128
                        nc.tensor.matmul(ps[:], bl, sqdeg_s[:, r0:r1],
                                         start=True, stop=False)
                        opened.add(b)
                nch_g = int(nch_gq[g].sum())
                g16_0 = segs[cur[0]][4] * 8
                idx_g = ipool.tile([128, nch_g * 8], I16, tag="idxs",
                                   name=f"ix_{rep}_{layer}_{g}")
                nc.sync.dma_start(idx_g[:], idx_d[:, g16_0:g16_0 + nch_g * 8])
                y_full = y_fulls[(rep, layer)]
                for qq in range(NQ):
                    nch = int(nch_gq[g, qq])
                    if nch == 0:
                        continue
                    ci0 = segs[cur[0]][4]
                    n_idx = nch * 128
                    n16 = n_idx // 16
                    off16 = ci0 * 8 - g16_0
                    mt = mpool.tile([128, nch, 128], BF16, tag="m",
                                    name=f"m_{rep}_{layer}_{g}_{qq}")
                    src = y_full[qq * 2 * SHARD_P:(qq + 1) * 2 * SHARD_P, :]
                    nc.gpsimd.dma_gather(
                        mt[:], src, idx_g[:, off16:off16 + n16],
                        n_idx, n_idx, D, single_packet=False,
                        queue_num=qrr % _NQUEUE)
                    qrr += 1
                    if not _NO_SBUILD:
                        # one batched is_equal for all chunks of this (g,q)
                        st_t = spool.tile([128, nch * 128], BF16, tag="s",
                                          name=f"s_{rep}_{layer}_{g}_{qq}")
                        seg_onehot(st_t, ci0, nch)
                    k = 0
                    while k < nch:
                        g2, q2, b2, nck, ci = segs[cur[0]]
                        assert g2 == g and q2 == qq and ci == ci0 + k
                        if not _NO_MM:
                            for u in range(nck):
                                first = b2 not in opened
                                opened.add(b2)
                                nc.tensor.matmul(
                                    psums[b2][:], mt[:, k + u, :],
                                    st_t[:, (k + u) * 128:(k + u + 1) * 128],
                                    start=first, stop=False)
                        k += nck
                        cur[0] += 1
                # close each block: self-loop identity matmul
                # (h = dinv*hTu, so the self message dinv^2*(hW) is just y)
                for b in blocks:
                    pq = b // BPP
                    pj = b - PSTART[pq]
                    y_q = hT[("y", layer, pq)]
                    nc.tensor.matmul(
                        psums[b][:], y_q[:, pj * 128:(pj + 1) * 128],
                        ident_s[:], start=(b not in opened), stop=True)
                for b in blocks:
                    pq = b // BPP
                    pj = b - PSTART[pq]
                    if (rep, layer, pq) not in hT:
                        hT[(rep, layer, pq)] = hpool.tile(
                            [128, PROWS[pq]], BF16, tag="h",
                            name=f"h_{rep}_{layer}_{pq}")
                    func = AF.Relu if layer < _NLAYERS - 1 else AF.Copy
                    h_blk = hT[(rep, layer, pq)][:, pj * 128:(pj + 1) * 128]
                    nc.scalar.activation(h_blk, psums[b][:], func)
                    if _FUSEPROJ and do_proj:
                        # fold this layer's projection term into the
                        # accumulator so hTu can be dropped after use
                        r0, r1 = b * 128, (b + 1) * 128
                        pf = ppy.tile([128, D_LAB], F32, tag="pf",
                                      name=f"pf_{rep}_{layer}_{b}")
                        nc.tensor.matmul(
                            pf[:], h_blk,
                            wp_s[:, layer * D_LAB:(layer + 1) * D_LAB],
                            start=True, stop=(layer != 0 or zero_bias))
                        if layer == 0 and not zero_bias:
                            r0, r1 = b * 128, (b + 1) * 128
                            nc.tensor.matmul(pf[:], sqdeg_s[:, r0:r1],
                                             bp_s[:], start=False, stop=True)
                        acc_sl = proj_acc[:, b * D_LAB:(b + 1) * D_LAB]
                        if layer == 0:
                            nc.vector.tensor_copy(acc_sl, pf[:])
                        else:
                            nc.vector.tensor_add(acc_sl, acc_sl, pf[:])

            def emit_projection_part(rep, qq):
                """out[t, l] = dinv[t]*(sum_i hTu_i^T Wp_i + sqrt(deg) bp)."""
                for j in range(PBLK[qq]):
                    b = PSTART[qq] + j
                    r0, r1 = b * 128, (b + 1) * 128
                    pf = ppy.tile([128, D_LAB], F32, tag="pf",
                                  name=f"pf_{rep}_{b}")
                    r0, r1 = b * 128, (b + 1) * 128
                    if not zero_bias:
                        nc.tensor.matmul(pf[:], sqdeg_s[:, r0:r1], bp_s[:],
                                         start=True, stop=False)
                    for i in range(3):
                        nc.tensor.matmul(
                            pf[:], hT[(i, qq)][:, j * 128:(j + 1) * 128],
                            wp_s[:, i * D_LAB:(i + 1) * D_LAB],
                            start=(i == 0 and zero_bias), stop=(i == 2))
                    fo = wpool.tile([128, D_LAB], F32, tag="fo")
                    nc.scalar.activation(fo[:], pf[:], AF.Copy,
                                         bias=0.0, scale=dinv_s[:, b:b + 1])
                    nc.sync.dma_start(out_d[r0:r1, :], fo[:])

            # part qq's hTu is complete after this aggregation group closes
            PART_READY = {(PSTART[p] + PBLK[p] - 1) // G: p for p in range(NQ)}
            do_proj = not (_SKIP_FINAL or _NLAYERS < 3)

            for rep in range(_REPEAT):
                if _FUSEPROJ and do_proj:
                    proj_acc = apool.tile([128, B * D_LAB], F32, tag="acc",
                                          name=f"acc_{rep}")
                for qq in range(NQ):
                    emit_transform_part(rep, 0, qq)
                emit_allgather(rep, 0)
                for layer in range(_NLAYERS):
                    if not _ILV and layer > 0:
                        for qq in range(NQ):
                            emit_transform_part(rep, layer, qq)
                        emit_allgather(rep, layer)
                    if _SKIP_AGG:
                        for qq in range(NQ):
                            h_q = hpool.tile([128, PROWS[qq]], BF16, tag="h",
                                             name=f"h_{rep}_{layer}_{qq}")
                            nc.vector.tensor_copy(h_q[:],
                                                  hT[("y", layer, qq)][:])
                            hT[(rep, layer, qq)] = h_q
                            if layer + 1 < _NLAYERS:
                                emit_transform_part(rep, layer + 1, qq)
                                if qq == NQ - 1:
                                    emit_allgather(rep, layer + 1)
                        continue
                    cur = [0]
                    for g in range(NG):
                        emit_agg_group(rep, layer, g, cur)
                        p = PART_READY.get(g)
                        if _ILV and p is not None:
                            # interleave next-layer transform (or projection)
                            # so its stage writes hide behind this aggregation
                            if layer + 1 < _NLAYERS:
                                emit_transform_part(rep, layer + 1, p)
                                if p == NQ - 1:
                                    emit_allgather(rep, layer + 1)
                            elif (layer == _NLAYERS - 1 and do_proj
                                  and not _FUSEPROJ):
                                emit_projection_part(rep, p)
                    assert cur[0] == len(segs)
                if not _ILV and do_proj and not _FUSEPROJ:
                    for qq in range(NQ):
                        emit_projection_part(rep, qq)
                if _FUSEPROJ and do_proj:
                    for b in range(B):
                        r0, r1 = b * 128, (b + 1) * 128
                        fo = wpool.tile([128, D_LAB], F32, tag="fo")
                        nc.scalar.activation(
                            fo[:], proj_acc[:, b * D_LAB:(b + 1) * D_LAB],
                            AF.Copy, bias=0.0, scale=dinv_s[:, b:b + 1])
                        nc.sync.dma_start(out_d[r0:r1, :], fo[:])

    nc.compile()
    return nc


_CACHE = {}


def _get_program(edge_index, zero_bias):
    key = (hash(np.asarray(edge_index).tobytes()), zero_bias)
    if key not in _CACHE:
        pre = _preprocess(edge_index)
        pre["zero_bias"] = zero_bias
        nc = _build(pre)
        _CACHE.clear()
        _CACHE[key] = (pre, nc)
    return _CACHE[key]


def prepare(feat, edge_index, W1, b1, W2, b2, W3, b3, Wp, bp):
    """Build (nc, in_maps) for the SPMD run."""
    feat = np.asarray(feat, np.float32)
    edge_index = np.asarray(edge_index, np.int32)
    W1, b1, W2, b2, W3, b3, Wp, bp = (np.asarray(a, np.float32)
                                      for a in (W1, b1, W2, b2, W3, b3, Wp, bp))
    zero_bias = not (np.any(b1) or np.any(b2) or np.any(b3) or np.any(bp))
    pre, nc = _get_program(edge_index, zero_bias)
    global LAST_PRE
    LAST_PRE = pre

    w_all = np.concatenate([W1, W2, W3], axis=1).astype(ml_dtypes.bfloat16)
    b_all = np.concatenate([b1, b2, b3]).reshape(1, 3 * D).astype(np.float32)
    wp_all = np.concatenate([Wp[:D], Wp[D:2 * D], Wp[2 * D:]],
                            axis=1).astype(ml_dtypes.bfloat16)
    iota = np.broadcast_to(np.arange(128, dtype=np.float32),
                           (128, 128)).astype(ml_dtypes.bfloat16)
    ident = np.eye(128, dtype=np.float32).astype(ml_dtypes.bfloat16)

    feat_p = np.zeros((NCORES, SHARD_P, D), np.float32)
    newpos = pre["newpos"]
    rows = np.repeat(np.arange(NCORES), SHARD)
    feat_p[rows, newpos.ravel()] = feat.reshape(NCORES * SHARD, D)
    featT = np.ascontiguousarray(
        feat_p.transpose(0, 2, 1)).astype(ml_dtypes.bfloat16)

    in_maps = []
    for c in range(NCORES):
        in_maps.append({
            "featT": featT[c],
            "idx": pre["idx"][c],
            "tgt": pre["tgt"][c],
            "dinv_col": pre["dinv_col"][c],
            "dinv2_col": pre["dinv2_col"][c],
            "sqdeg_row": pre["sqdeg_row"][c],
            "w_all": w_all, "b_all": b_all,
            "wp_all": wp_all, "bp": bp.reshape(1, D_LAB).astype(np.float32),
            "iota": iota, "ident": ident,
        })
    return nc, in_maps


def kernel(**inputs):
    nc, in_maps = prepare(**inputs)
    trace = bool(int(os.environ.get("GCN_TRACE", "0")))
    res = bass_utils.run_bass_kernel_spmd(nc, in_maps,
                                          core_ids=list(range(NCORES)),
                                          trace=trace)
    global LAST_RESULTS
    LAST_RESULTS = res
    newpos = LAST_PRE["newpos"]
    out = np.empty((N_NODES, D_LAB), np.float32)
    for c in range(NCORES):
        out[c * SHARD:(c + 1) * SHARD] = res.results[c]["out"][newpos[c]]
    return out


LAST_RESULTS = None
LAST_PRE = None



# revision 20
# speedup vs baseline: 1.5809x; 1.1349x over previous
"""GCN (3-layer + linear head) Trainium2 Bass kernel, sharded over 8 NeuronCores.

v2 strategy (vertex partitioning per the sharding hint, all-bf16 datapath):
 - Nodes sharded contiguously: core c owns nodes [c*12500, (c+1)*12500),
   padded to 12544 = 98 blocks of 128 rows.
 - Per layer: transform y = dinv^k * (h @ W) runs directly off the
   transposed hidden state hT kept in SBUF (no PE transposes anywhere);
   y shards are AllGathered part-wise (4 partial collectives per layer,
   bf16) so the halo exchange overlaps the transform and aggregation.
 - Aggregation gathers per-edge source rows (dma_gather, 256B bf16 rows)
   and realizes the segment-sum as one-hot matmuls accumulating into
   per-target-block PSUM banks. One-hot builds are batched per segment
   with a single broadcast-AP is_equal on the vector engine.
 - Self-loop messages skip the gather entirely: they are applied as a
   diag(dinv) matmul against the local y block in SBUF.
 - The hidden state is stored unscaled (relu(dinv*(x))=dinv*relu(x));
   dinv factors fold into the next transform (dinv^2) and the final
   projection (dinv), both per-partition activation scales.
 - Host does integer index prep only: degree counts, edge sorting by
   (core, group, part, block, source), structural chunk padding shared
   across cores so all 8 run one SPMD instruction stream.
"""
import os
import sys

sys.path.insert(0, "/opt/trn_rl_repo")

import numpy as np
import ml_dtypes

_NLAYERS = int(os.environ.get("GCN_NLAYERS", "3"))
_SKIP_AGG = bool(int(os.environ.get("GCN_SKIP_AGG", "0")))
_SKIP_FINAL = bool(int(os.environ.get("GCN_SKIP_FINAL", "0")))
_NO_MM = bool(int(os.environ.get("GCN_NO_MM", "0")))
_NO_SBUILD = bool(int(os.environ.get("GCN_NO_SBUILD", "0")))
_ONECORE = bool(int(os.environ.get("GCN_ONECORE", "0")))
_REPEAT = int(os.environ.get("GCN_REPEAT", "1"))
_NAG = int(os.environ.get("GCN_NAG", "4"))       # parts per AllGather split
_NO_AG = bool(int(os.environ.get("GCN_NO_AG", "0")))
_ILV = bool(int(os.environ.get("GCN_INTERLEAVE", "0")))
_FUSEPROJ = bool(int(os.environ.get("GCN_FUSEPROJ", "1")))
_NQUEUE = int(os.environ.get("GCN_NQUEUE", "4"))
_MPOOL = int(os.environ.get("GCN_MPOOL", "14"))
_PPA = int(os.environ.get("GCN_PPA", str(max(4, G))))
_SPOOL = int(os.environ.get("GCN_SPOOL", "3"))
_YPOOL = int(os.environ.get("GCN_YPOOL", "8"))
_XREP = bool(int(os.environ.get("GCN_XREP", "1")))

import concourse.bacc as bacc
import concourse.mybir as mybir
import concourse.tile as tile
from concourse import bass_utils
from concourse.bass import AP
from concourse.library_config import mlp

# Problem constants (hardcoded per harness contract).
N_NODES = 100000
N_EDGES = 1600000
D = 128
D_LAB = 10
NCORES = 8
SHARD = 12500
B = 104                   # target blocks per core (128 rows each, ~120 used)
SHARD_P = B * 128         # 13312 padded rows per core
G = int(os.environ.get("GCN_G", "4"))  # target blocks per agg group
NG = -(-B // G)
BPP = B // 4              # blocks per transform part (26)
PBLK = [BPP] * 4
PSTART = [0, BPP, 2 * BPP, 3 * BPP]
PROWS = [p * 128 for p in PBLK]
NQ = 4                    # gather source groups (core pairs)

F32 = mybir.dt.float32
BF16 = mybir.dt.bfloat16
I16 = mybir.dt.int16
AF = mybir.ActivationFunctionType
ALU = mybir.AluOpType

PAD_TGT = 200.0           # bf16-exact, outside [0,128): padded slots match nothing


QOFF = [0, PROWS[0], 2 * PROWS[0], 3 * PROWS[0]]  # part row offsets in stage


def _node_layout(src, tgt):
    """Per-core node->slot permutation: stratified deal + repair so every
    (source-pair, target-block) cell holds <= 512 edges (C=4 chunks), which
    eliminates nearly all structural SPMD padding."""
    pair = (src // SHARD) // 2
    c_t_all = tgt // SHARD
    newpos = np.zeros((NCORES, SHARD), np.int64)
    for c in range(NCORES):
        m = c_t_all == c
        tl0 = tgt[m] % SHARD
        v = np.zeros((SHARD, 4), np.int64)
        np.add.at(v, (tl0, pair[m]), 1)
        order = np.lexsort((v[:, 3], v[:, 2], v[:, 1], v[:, 0]))
        node_blk = np.zeros(SHARD, np.int64)
        node_blk[order] = np.arange(SHARD) % B
        for _ in range(3):
            bs = np.stack([np.bincount(node_blk, weights=v[:, qq],
                                       minlength=B)
                           for qq in range(4)], axis=1).astype(np.int64)
            cap = np.bincount(node_blk, minlength=B)
            over = np.argwhere(bs > 512)
            if len(over) == 0:
                break
            for b, qq in over:
                need = bs[b, qq] - 512
                nodes = np.where(node_blk == b)[0]
                cand = nodes[np.argsort(-(2 * v[nodes, qq]
                                          - v[nodes].sum(1)))]
                for n in cand:
                    if need <= 0 or v[n, qq] == 0:
                        break
                    slack = 512 - bs[:, qq]
                    slack[cap >= 126] = -1 << 30
                    b2 = int(np.argmax(slack))
                    if slack[b2] < v[n, qq]:
                        break
                    node_blk[n] = b2
                    bs[b] -= v[n]
                    bs[b2] += v[n]
                    cap[b] -= 1
                    cap[b2] += 1
                    need -= v[n, qq]
        o2 = np.argsort(node_blk, kind="stable")
        starts = np.searchsorted(node_blk[o2], np.arange(B))
        ranks = np.arange(SHARD) - starts[node_blk[o2]]
        within = np.zeros(SHARD, np.int64)
        within[o2] = ranks
        assert within.max() < 128
        newpos[c] = node_blk * 128 + within
    return newpos


def _stage_row_of(slot):
    """Stage row of a node at padded local slot (transposed part layout)."""
    rq = slot // PROWS[0]
    qoff = np.asarray(QOFF, np.int64)[rq]
    pstart = np.asarray(PSTART, np.int64)[rq]
    return qoff + (slot % 128) * BPP + (slot // 128 - pstart)


def _preprocess(edge_index):
    """Host-side integer/index prep. Returns per-core arrays + shared structure.

    One AllGather per layer — the gathered buffer is core-major
    [core, stage_row]; gather source groups are core PAIRS (idx16 =
    (c_s % 2) * SHARD_P + stage_row <= 26623, int16-safe) with the
    dma_gather base AP offset selecting the pair."""
    src = np.asarray(edge_index[0], dtype=np.int64)
    tgt = np.asarray(edge_index[1], dtype=np.int64)

    newpos = _node_layout(src, tgt)

    # degree includes the self-loop (handled on-device via diag matmul)
    deg = (np.bincount(tgt, minlength=N_NODES) + 1).astype(np.float64)

    c_t = tgt // SHARD
    tl = newpos[c_t, tgt % SHARD]
    blk = tl // 128
    grp = blk // G
    c_s = src // SHARD
    r = newpos[c_s, src % SHARD]
    stage_row = _stage_row_of(r)
    q = c_s // 2  # gather source group = core pair
    qrel = (c_s % 2) * SHARD_P + stage_row

    order = np.lexsort((qrel, blk, q, grp, c_t))
    c_t, tl, blk, q, qrel = c_t[order], tl[order], blk[order], q[order], qrel[order]

    # segment = (core, g, q, blk); structural chunk count = max over cores
    seg_key = ((c_t * NG + (blk // G)) * NQ + q) * B + blk
    nseg = NCORES * NG * NQ * B
    counts = np.bincount(seg_key, minlength=nseg).reshape(NCORES, NG, NQ, B)
    C = (-(-counts // 128)).max(axis=0)  # [NG, NQ, B]

    tot_chunks = int(C.sum())
    TOT = tot_chunks * 128
    idx_all = np.zeros((NCORES, TOT), dtype=np.int16)
    tgt_all = np.full((NCORES, TOT), PAD_TGT, dtype=np.float32)

    seg_starts = np.zeros(nseg + 1, dtype=np.int64)
    np.cumsum(np.bincount(seg_key, minlength=nseg), out=seg_starts[1:])

    segs = []       # (g, q, b, nch, ci0) in emission order
    nch_gq = np.zeros((NG, NQ), dtype=np.int64)
    off = 0
    for g in range(NG):
        for qq in range(NQ):
            for b in range(g * G, min((g + 1) * G, B)):
                nch = int(C[g, qq, b])
                if nch == 0:
                    continue
                segs.append((g, qq, b, nch, off // 128))
                nch_gq[g, qq] += nch
                for c in range(NCORES):
                    k = ((c * NG + g) * NQ + qq) * B + b
                    s0, s1 = seg_starts[k], seg_starts[k + 1]
                    n = s1 - s0
                    idx_all[c, off:off + n] = qrel[s0:s1]
                    tgt_all[c, off:off + n] = (tl[s0:s1] - b * 128)
                off += nch * 128
    assert off == TOT

    idx_wrapped = np.stack([
        np.tile(a.reshape(-1, 16).T, (8, 1)) for a in idx_all])
    tgt_tiles = np.ascontiguousarray(
        tgt_all.reshape(NCORES, tot_chunks, 128).transpose(0, 2, 1)
    ).astype(ml_dtypes.bfloat16)

    deg_p = np.ones((NCORES, SHARD_P), dtype=np.float64)
    rows = np.repeat(np.arange(NCORES), SHARD)
    deg_p[rows, newpos.ravel()] = deg
    dinv = 1.0 / np.sqrt(deg_p)
    dinv_col = np.ascontiguousarray(
        dinv.reshape(NCORES, B, 128).transpose(0, 2, 1)).astype(np.float32)
    dinv2_col = np.ascontiguousarray(
        (1.0 / deg_p).reshape(NCORES, B, 128).transpose(0, 2, 1)).astype(np.float32)
    sqdeg_row = np.sqrt(deg_p).reshape(NCORES, 1, SHARD_P).astype(np.float32)

    return dict(idx=idx_wrapped, tgt=tgt_tiles, dinv_col=dinv_col,
                dinv2_col=dinv2_col, sqdeg_row=sqdeg_row,
                segs=segs, nch_gq=nch_gq, tot_chunks=tot_chunks, TOT=TOT,
                newpos=newpos)


def _build(pre):
    """Build the Bass/Tile program (one SPMD NEFF for all 8 cores)."""
    zero_bias = bool(pre.get("zero_bias", False))
    TOT = pre["TOT"]
    tot_chunks = pre["tot_chunks"]
    nch_gq = pre["nch_gq"]
    segs = pre["segs"]

    nc = bacc.Bacc("TRN2", target_bir_lowering=False, debug=False,
                   num_devices=1 if _ONECORE else NCORES,
                   num_swdge_queues=_NQUEUE)

    featT_d = nc.dram_tensor("featT", [D, SHARD_P], BF16, kind="ExternalInput")
    idx_d = nc.dram_tensor("idx", [128, TOT // 16], I16, kind="ExternalInput")
    tgt_d = nc.dram_tensor("tgt", [128, tot_chunks], BF16, kind="ExternalInput")
    dinv_d = nc.dram_tensor("dinv_col", [128, B], F32, kind="ExternalInput")
    dinv2_d = nc.dram_tensor("dinv2_col", [128, B], F32, kind="ExternalInput")
    sqdeg_d = nc.dram_tensor("sqdeg_row", [1, SHARD_P], F32, kind="ExternalInput")
    w_d = nc.dram_tensor("w_all", [128, 3 * D], BF16, kind="ExternalInput")
    b_d = nc.dram_tensor("b_all", [1, 3 * D], F32, kind="ExternalInput")
    wp_d = nc.dram_tensor("wp_all", [128, 3 * D_LAB], BF16, kind="ExternalInput")
    bp_d = nc.dram_tensor("bp", [1, D_LAB], F32, kind="ExternalInput")
    iota_d = nc.dram_tensor("iota", [128, 128], BF16, kind="ExternalInput")
    ident_d = nc.dram_tensor("ident", [128, 128], BF16, kind="ExternalInput")

    out_d = nc.dram_tensor("out", [SHARD_P, D_LAB], F32, kind="ExternalOutput")

    with tile.TileContext(nc) as tc:
        with (
            tc.tile_pool(name="const", bufs=1) as cpool,
            tc.tile_pool(name="work", bufs=3) as wpool,
            tc.tile_pool(name="hstate", bufs=(2 if _FUSEPROJ else 3) * NQ) as hpool,
            tc.tile_pool(name="ystate", bufs=_YPOOL) as ypool,
            tc.tile_pool(name="projacc", bufs=2) as apool,
            tc.tile_pool(name="sbuild", bufs=_SPOOL) as spool,
            tc.tile_pool(name="mtiles", bufs=_MPOOL) as mpool,
            tc.tile_pool(name="itiles", bufs=3) as ipool,
            tc.tile_pool(name="psum_a", bufs=_PPA, space="PSUM") as ppa,
            tc.tile_pool(name="psum_y", bufs=2, space="PSUM") as ppy,
            tc.tile_pool(name="dram", bufs=1, space="DRAM") as dpool,
        ):
            nc.gpsimd.load_library(mlp)

            # ---- constants ----
            tgt_s = cpool.tile([128, tot_chunks], BF16)
            iota_s = cpool.tile([128, 128], BF16)
            ident_s = cpool.tile([128, 128], BF16)
            w_s = cpool.tile([128, 3 * D], BF16)
            b_s = cpool.tile([1, 3 * D], F32)
            wp_s = cpool.tile([128, 3 * D_LAB], BF16)
            bp_s = cpool.tile([1, D_LAB], F32)
            dinv_s = cpool.tile([128, B], F32)
            dinv2_s = cpool.tile([128, B], F32)
            sqdeg_s = None
            if not zero_bias:
                sqdeg_s = cpool.tile([1, SHARD_P], F32)
                nc.sync.dma_start(sqdeg_s[:], sqdeg_d[:])

            nc.sync.dma_start(tgt_s[:], tgt_d[:])
            nc.sync.dma_start(iota_s[:], iota_d[:])
            nc.sync.dma_start(ident_s[:], ident_d[:])
            nc.sync.dma_start(w_s[:], w_d[:])
            nc.sync.dma_start(b_s[:], b_d[:])
            nc.sync.dma_start(wp_s[:], wp_d[:])
            nc.sync.dma_start(bp_s[:], bp_d[:])
            nc.sync.dma_start(dinv_s[:], dinv_d[:])
            nc.sync.dma_start(dinv2_s[:], dinv2_d[:])

            # ---- internal DRAM: one AllGather landing buffer per (rep,layer)
            y_fulls = {}
            y_stage = {}
            for rep in range(_REPEAT):
                for layer in range(_NLAYERS):
                    y_fulls[(rep, layer)] = dpool.tile(
                        [NCORES * SHARD_P, D], BF16,
                        addr_space=os.environ.get("GCN_YPART_SPACE",
                                                  "Shared"),
                        name=f"y_full_{rep}_{layer}")
                    y_stage[(rep, layer)] = dpool.tile(
                        [SHARD_P, D], BF16,
                        name=f"y_stage_{rep}_{layer}")

            hT = {}          # (layer, qq) -> SBUF tile [128 f, PROWS[q] cols]
            qrr = 0          # gather queue round-robin

            def seg_onehot(st_t, ci0, nch):
                """st[:, k*128+j] = (iota[:, j] == tgt[:, ci0+k]), batched."""
                o = AP(st_t[:, :].tensor, st_t[:, :].offset,
                       [st_t[:, :].ap[0], [128, nch], [1, 128]])
                i1 = AP(iota_s[:, :].tensor, iota_s[:, :].offset,
                        [iota_s[:, :].ap[0], [0, nch], [1, 128]])
                t2 = tgt_s[:, ci0:ci0 + nch]
                i2 = AP(t2.tensor, t2.offset, [t2.ap[0], [1, nch], [0, 128]])
                nc.vector.tensor_tensor(o, i1, i2, ALU.is_equal)

            def emit_transform_part(rep, layer, qq):
                """y = dinv^k * (hT_prev^T @ W) for part qq -> stage rows."""
                wl = w_s[:, layer * D:(layer + 1) * D]
                y_q = ypool.tile([128, PROWS[qq]], BF16, tag="ysb",
                                 name=f"y_{rep}_{layer}_{qq}")
                for j in range(PBLK[qq]):
                    b = PSTART[qq] + j
                    if layer == 0:
                        fb = wpool.tile([128, 128], BF16, tag="fb")
                        nc.sync.dma_start(
                            fb[:], featT_d[:, b * 128:(b + 1) * 128])
                        stat = fb[:]
                    else:
                        stat = hT[(rep, layer - 1, qq)][:, j * 128:(j + 1) * 128]
                    yp = ppy.tile([128, 128], F32, tag="ty")
                    nc.tensor.matmul(yp[:], stat, wl, start=True, stop=True)
                    scale = (dinv_s if layer == 0 else dinv2_s)[:, b:b + 1]
                    nc.scalar.activation(y_q[:, j * 128:(j + 1) * 128],
                                         yp[:], AF.Copy,
                                         bias=0.0, scale=scale)
                stage = y_stage[(rep, layer)]
                nc.sync.dma_start(
                    stage[QOFF[qq]:QOFF[qq] + PROWS[qq], :], y_q[:])
                hT[("y", layer, qq)] = y_q

            def emit_allgather(rep, layer):
                if _ONECORE:
                    nc.sync.dma_start(
                        y_fulls[(rep, layer)][0:SHARD_P, :],
                        y_stage[(rep, layer)][:])
                    return
                if _NO_AG:
                    return
                nc.gpsimd.collective_compute(
                    "AllGather", ALU.bypass,
                    replica_groups=[list(range(NCORES))],
                    ins=[y_stage[(rep, layer)].opt()],
                    outs=[y_fulls[(rep, layer)].opt()],
                )

            def emit_agg_group(rep, layer, g, cur):
                nonlocal qrr
                bl = b_s[:, layer * D:(layer + 1) * D]
                blocks = list(range(g * G, min((g + 1) * G, B)))
                psums = {}
                opened = set()
                for b in blocks:
                    ps = ppa.tile([128, 128], F32, tag="agg",
                                  name=f"ps_{rep}_{layer}_{b}")
                    psums[b] = ps
                    if not zero_bias:
                        # open the accumulation group with the bias rank-1
                        r0, r1 = b * 128, (b + 1) * 128
                        nc.tensor.matmul(ps[:], bl, sqdeg_s[:, r0:r1],
                                         start=True, stop=False)
                        opened.add(b)
                nch_g = int(nch_gq[g].sum())
                g16_0 = segs[cur[0]][4] * 8
                idx_g = ipool.tile([128, nch_g * 8], I16, tag="idxs",
                                   name=f"ix_{rep}_{layer}_{g}")
                nc.sync.dma_start(idx_g[:], idx_d[:, g16_0:g16_0 + nch_g * 8])
                y_full = y_fulls[(rep, layer)]
                for qq in range(NQ):
                    nch = int(nch_gq[g, qq])
                    if nch == 0:
                        continue
                    ci0 = segs[cur[0]][4]
                    n_idx = nch * 128
                    n16 = n_idx // 16
                    off16 = ci0 * 8 - g16_0
                    mt = mpool.tile([128, nch, 128], BF16, tag="m",
                                    name=f"m_{rep}_{layer}_{g}_{qq}")
                    src = y_full[qq * 2 * SHARD_P:(qq + 1) * 2 * SHARD_P, :]
                    nc.gpsimd.dma_gather(
                        mt[:], src, idx_g[:, off16:off16 + n16],
                        n_idx, n_idx, D, single_packet=False,
                        queue_num=qrr % _NQUEUE)
                    qrr += 1
                    if not _NO_SBUILD:
                        # one batched is_equal for all chunks of this (g,q)
                        st_t = spool.tile([128, nch * 128], BF16, tag="s",
                                          name=f"s_{rep}_{layer}_{g}_{qq}")
                        seg_onehot(st_t, ci0, nch)
                    k = 0
                    while k < nch:
                        g2, q2, b2, nck, ci = segs[cur[0]]
                        assert g2 == g and q2 == qq and ci == ci0 + k
                        if not _NO_MM:
                            for u in range(nck):
                                first = b2 not in opened
                                opened.add(b2)
                                nc.tensor.matmul(
                                    psums[b2][:], mt[:, k + u, :],
                                    st_t[:, (k + u) * 128:(k + u + 1) * 128],
                                    start=first, stop=False)
                        k += nck
                        cur[0] += 1
                # close each block: self-loop identity matmul
                # (h = dinv*hTu, so the self message dinv^2*(hW) is just y)
                for b in blocks:
                    pq = b // BPP
                    pj = b - PSTART[pq]
                    y_q = hT[("y", layer, pq)]
                    nc.tensor.matmul(
                        psums[b][:], y_q[:, pj * 128:(pj + 1) * 128],
                        ident_s[:], start=(b not in opened), stop=True)
                for b in blocks:
                    pq = b // BPP
                    pj = b - PSTART[pq]
                    if (rep, layer, pq) not in hT:
                        hT[(rep, layer, pq)] = hpool.tile(
                            [128, PROWS[pq]], BF16, tag="h",
                            name=f"h_{rep}_{layer}_{pq}")
                    func = AF.Relu if layer < _NLAYERS - 1 else AF.Copy
                    h_blk = hT[(rep, layer, pq)][:, pj * 128:(pj + 1) * 128]
                    nc.scalar.activation(h_blk, psums[b][:], func)
                    if _FUSEPROJ and do_proj:
                        # fold this layer's projection term into the
                        # accumulator so hTu can be dropped after use
                        r0, r1 = b * 128, (b + 1) * 128
                        pf = ppy.tile([128, D_LAB], F32, tag="pf",
                                      name=f"pf_{rep}_{layer}_{b}")
                        nc.tensor.matmul(
                            pf[:], h_blk,
                            wp_s[:, layer * D_LAB:(layer + 1) * D_LAB],
                            start=True, stop=(layer != 0 or zero_bias))
                        if layer == 0 and not zero_bias:
                            r0, r1 = b * 128, (b + 1) * 128
                            nc.tensor.matmul(pf[:], sqdeg_s[:, r0:r1],
                                             bp_s[:], start=False, stop=True)
                        acc_sl = proj_acc[:, b * D_LAB:(b + 1) * D_LAB]
                        if layer == 0:
                            nc.vector.tensor_copy(acc_sl, pf[:])
                        else:
                            nc.vector.tensor_add(acc_sl, acc_sl, pf[:])

            def emit_projection_part(rep, qq):
                """out[t, l] = dinv[t]*(sum_i hTu_i^T Wp_i + sqrt(deg) bp)."""
                for j in range(PBLK[qq]):
                    b = PSTART[qq] + j
                    r0, r1 = b * 128, (b + 1) * 128
                    pf = ppy.tile([128, D_LAB], F32, tag="pf",
                                  name=f"pf_{rep}_{b}")
                    r0, r1 = b * 128, (b + 1) * 128
                    if not zero_bias:
                        nc.tensor.matmul(pf[:], sqdeg_s[:, r0:r1], bp_s[:],
                                         start=True, stop=False)
                    for i in range(3):
                        nc.tensor.matmul(
                            pf[:], hT[(i, qq)][:, j * 128:(j + 1) * 128],
                            wp_s[:, i * D_LAB:(i + 1) * D_LAB],
                            start=(i == 0 and zero_bias), stop=(i == 2))
                    fo = wpool.tile([128, D_LAB], F32, tag="fo")
                    nc.scalar.activation(fo[:], pf[:], AF.Copy,
                                         bias=0.0, scale=dinv_s[:, b:b + 1])
                    nc.sync.dma_start(out_d[r0:r1, :], fo[:])

            # part qq's hTu is complete after this aggregation group closes
            PART_READY = {(PSTART[p] + PBLK[p] - 1) // G: p for p in range(NQ)}
            do_proj = not (_SKIP_FINAL or _NLAYERS < 3)

            for rep in range(_REPEAT):
                if _FUSEPROJ and do_proj:
                    proj_acc = apool.tile([128, B * D_LAB], F32, tag="acc",
                                          name=f"acc_{rep}")
                if rep == 0 or not _XREP:
                    # (later reps' layer-0 transform+AG are emitted inside
                    # the previous rep's last agg loop to hide the AllGather)
                    for qq in range(NQ):
                        emit_transform_part(rep, 0, qq)
                    emit_allgather(rep, 0)
                for layer in range(_NLAYERS):
                    if not _ILV and layer > 0:
                        for qq in range(NQ):
                            emit_transform_part(rep, layer, qq)
                        emit_allgather(rep, layer)
                    if _SKIP_AGG:
                        for qq in range(NQ):
                            h_q = hpool.tile([128, PROWS[qq]], BF16, tag="h",
                                             name=f"h_{rep}_{layer}_{qq}")
                            nc.vector.tensor_copy(h_q[:],
                                                  hT[("y", layer, qq)][:])
                            hT[(rep, layer, qq)] = h_q
                            if layer + 1 < _NLAYERS:
                                emit_transform_part(rep, layer + 1, qq)
                                if qq == NQ - 1:
                                    emit_allgather(rep, layer + 1)
                        continue
                    cur = [0]
                    for g in range(NG):
                        emit_agg_group(rep, layer, g, cur)
                        p = PART_READY.get(g)
                        if p is not None:
                            if _ILV and layer + 1 < _NLAYERS:
                                # interleave next-layer transform so its
                                # stage writes hide behind this aggregation
                                emit_transform_part(rep, layer + 1, p)
                                if p == NQ - 1:
                                    emit_allgather(rep, layer + 1)
                            elif (_ILV and layer == _NLAYERS - 1 and do_proj
                                  and not _FUSEPROJ):
                                emit_projection_part(rep, p)
                            if (_XREP and layer == _NLAYERS - 1
                                    and rep + 1 < _REPEAT):
                                # next rep's layer-0 transform+AG overlap
                                # this rep's tail aggregation
                                emit_transform_part(rep + 1, 0, p)
                                if p == NQ - 1:
                                    emit_allgather(rep + 1, 0)
                    assert cur[0] == len(segs)
                if not _ILV and do_proj and not _FUSEPROJ:
                    for qq in range(NQ):
                        emit_projection_part(rep, qq)
                if _FUSEPROJ and do_proj:
                    for b in range(B):
                        r0, r1 = b * 128, (b + 1) * 128
                        fo = wpool.tile([128, D_LAB], F32, tag="fo")
                        nc.scalar.activation(
                            fo[:], proj_acc[:, b * D_LAB:(b + 1) * D_LAB],
                            AF.Copy, bias=0.0, scale=dinv_s[:, b:b + 1])
                        nc.sync.dma_start(out_d[r0:r1, :], fo[:])

    nc.compile()
    return nc


_CACHE = {}


def _get_program(edge_index, zero_bias):
    key = (hash(np.asarray(edge_index).tobytes()), zero_bias)
    if key not in _CACHE:
        pre = _preprocess(edge_index)
        pre["zero_bias"] = zero_bias
        nc = _build(pre)
        _CACHE.clear()
        _CACHE[key] = (pre, nc)
    return _CACHE[key]


def prepare(feat, edge_index, W1, b1, W2, b2, W3, b3, Wp, bp):
    """Build (nc, in_maps) for the SPMD run."""
    feat = np.asarray(feat, np.float32)
    edge_index = np.asarray(edge_index, np.int32)
    W1, b1, W2, b2, W3, b3, Wp, bp = (np.asarray(a, np.float32)
                                      for a in (W1, b1, W2, b2, W3, b3, Wp, bp))
    zero_bias = not (np.any(b1) or np.any(b2) or np.any(b3) or np.any(bp))
    pre, nc = _get_program(edge_index, zero_bias)
    global LAST_PRE
    LAST_PRE = pre

    w_all = np.concatenate([W1, W2, W3], axis=1).astype(ml_dtypes.bfloat16)
    b_all = np.concatenate([b1, b2, b3]).reshape(1, 3 * D).astype(np.float32)
    wp_all = np.concatenate([Wp[:D], Wp[D:2 * D], Wp[2 * D:]],
                            axis=1).astype(ml_dtypes.bfloat16)
    iota = np.broadcast_to(np.arange(128, dtype=np.float32),
                           (128, 128)).astype(ml_dtypes.bfloat16)
    ident = np.eye(128, dtype=np.float32).astype(ml_dtypes.bfloat16)

    feat_p = np.zeros((NCORES, SHARD_P, D), np.float32)
    newpos = pre["newpos"]
    rows = np.repeat(np.arange(NCORES), SHARD)
    feat_p[rows, newpos.ravel()] = feat.reshape(NCORES * SHARD, D)
    featT = np.ascontiguousarray(
        feat_p.transpose(0, 2, 1)).astype(ml_dtypes.bfloat16)

    in_maps = []
    for c in range(NCORES):
        in_maps.append({
            "featT": featT[c],
            "idx": pre["idx"][c],
            "tgt": pre["tgt"][c],
            "dinv_col": pre["dinv_col"][c],
            "dinv2_col": pre["dinv2_col"][c],
            "sqdeg_row": pre["sqdeg_row"][c],
            "w_all": w_all, "b_all": b_all,
            "wp_all": wp_all, "bp": bp.reshape(1, D_LAB).astype(np.float32),
            "iota": iota, "ident": ident,
        })
    return nc, in_maps


def kernel(**inputs):
    nc, in_maps = prepare(**inputs)
    trace = bool(int(os.environ.get("GCN_TRACE", "0")))
    res = bass_utils.run_bass_kernel_spmd(nc, in_maps,
                                          core_ids=list(range(NCORES)),
                                          trace=trace)
    global LAST_RESULTS
    LAST_RESULTS = res
    newpos = LAST_PRE["newpos"]
    out = np.empty((N_NODES, D_LAB), np.float32)
    for c in range(NCORES):
        out[c * SHARD:(c + 1) * SHARD] = res.results[c]["out"][newpos[c]]
    return out


LAST_RESULTS = None
LAST_PRE = None

